# revision 15
# baseline (speedup 1.0000x reference)
"""Trainium2 Bass kernel for 3D volume attention (b=2, x=y=z=16, c=64,
heads=4, dim_head=32, qk-standardize over sequence, scale=16).

Sharding: batch*heads = 8 (b,h) pairs -> 8 NeuronCores, one pair per core.
Host pre-transposes x and pre-slices per-head weights; host sums the 4
head-partials per batch (pure unshard-reduce) and reshapes.

Per-core pipeline (s=4096, d=32). v2 rewrite of the two-pass softmax
kernel, tuned from a perfetto trace of v1 (307us):
  prologue: standardization stats computed via tiny PE matmuls on the
            Gram matrix G = [x|1]^T [x|1] (sumsq_d = w_d^T G w_d,
            mean from the ones column) instead of serial ACT Square
            passes; rsqrt via exp(-0.5 ln v) + one Newton step so the
            WHOLE kernel uses only the natural_log_exp ACT table set
            (no table thrash); projections drain PSUM directly to the
            standardized bf16 qA (ACT) / kA (DVE) replicas; f32-hat
            rows 0:32 drained separately for the hi/lo residuals
            (gpsimd subtract); input DMA chunked so the PE starts
            early; bf16 v^T/identity transposes.
  pass A  (S[i,j], 2-way row-tiled concurrent matmul pairs): per-quarter
          DVE reduce_max; chunk 0 peeled with a dedicated 4-deep PSUM
          ring, quarters 0,1 exact max on DVE + quarters 2,3 lse bound
          on ACT (16*ln(sum exp(s/16)) - 40 >= max-40).
  pass B  (S^T[j,i]): K=97 bf16 matmul ([khi;khi;klo;-1] x
          [qhi;qlo;qhi;mhat]) -> ACT exp -> bf16 P^T.
  AV:     2-way COLUMN-TILED concurrent pairs: even j-blocks accumulate
          P^T @ [v|1] into PSUM partitions 0:33 (tile_position (0,0)),
          odd j-blocks into partitions 64:97 (tile_position (0,64));
          the epilogue adds the halves. Halves the AV stream time.
  out:    per chunk: DVE add of the two AV halves -> SBUF, project with
          [w_out_h ; b_out/4], DVE copy, DMA out. Host divides by the
          returned softmax denominator during unshard.

Software pipelining: AV lags exp by two iterations (pairs), pass A for
chunk c+1 interleaved one quarter per pass-B iteration. PSUM: 3x[128,1024]
shared ring + [128,1024] col-tiled AV accumulator (8 banks exactly).
"""
import os
import sys
from contextlib import ExitStack

import numpy as np

_PROBLEM_DIR = os.path.dirname(os.path.abspath(__file__))
if _PROBLEM_DIR not in sys.path:
    sys.path.insert(0, _PROBLEM_DIR)

import concourse.bass as bass
import concourse.tile as tile
from concourse import bacc, mybir
from concourse.bass_utils import run_bass_kernel_spmd

F32 = mybir.dt.float32
F32R = mybir.dt.float32r
BF16 = mybir.dt.bfloat16
AF = mybir.ActivationFunctionType
ALU = mybir.AluOpType

HEADS = 4
DH = 32          # dim head
CIN = 64         # input channels
S = 4096         # sequence (16^3)
SCALE = 16.0
EPS = 1e-5
NB = S // 128    # 32 j blocks
NCH = 4          # i chunks
CHUNK = 1024
KP = 97          # 3*32 pair rows + 1 aug row

_compiled = None
STAGE = int(os.environ.get("STAGE", "4"))  # 1=prologue 2=+peel 3=+main-loop 4=full


def _build():
    nc = bacc.Bacc("TRN2", target_bir_lowering=False, debug=False, num_devices=8)
    xT_d = nc.dram_tensor("xT", [CIN, S], F32R, kind="ExternalInput").ap()
    xa_d = nc.dram_tensor("xa", [128, NB, CIN + 1], BF16, kind="ExternalInput").ap()
    wq_d = nc.dram_tensor("wq", [CIN, 128], F32R, kind="ExternalInput").ap()
    wk_d = nc.dram_tensor("wk", [CIN, 128], F32R, kind="ExternalInput").ap()
    wqf_d = nc.dram_tensor("wqf", [CIN, 128], F32, kind="ExternalInput").ap()
    wkf_d = nc.dram_tensor("wkf", [CIN, 128], F32, kind="ExternalInput").ap()
    wv_d = nc.dram_tensor("wv", [CIN, DH], F32R, kind="ExternalInput").ap()
    wo_d = nc.dram_tensor("wo", [DH + 1, CIN], F32R, kind="ExternalInput").ap()
    out_d = nc.dram_tensor("out", [CIN, S], F32, kind="ExternalOutput").ap()
    # softmax denominator per column; host divides during unshard (standard
    # split-softmax partial combination)
    l_d = nc.dram_tensor("ldenom", [1, S], F32R, kind="ExternalOutput").ap()

    with tile.TileContext(nc) as tc, ExitStack() as ctx:
        per = ctx.enter_context(tc.tile_pool(name="per", bufs=1))

        # ---- persistent SBUF ----
        wo_r = per.tile([97, CIN], F32R)  # wo at rows 0:33 AND 64:97
        qA = per.tile([128, S], BF16)          # 4 replicated bands of qhat*16
        kA = per.tile([128, S], BF16)          # 4 replicated bands of khat
        kP = per.tile([KP, S], BF16)           # [khi; khi; klo; -1]
        vaug = per.tile([128, NB, DH + 1], BF16)   # per j-block [v | 1]
        qPc = [per.tile([KP, CHUNK], BF16, name=f"qPc{c}") for c in range(NCH)]
        mcolT = [per.tile([128, 8], F32, name=f"mcolT{c}") for c in range(NCH)]
        neg1 = per.tile([128, DH], BF16)
        identb = per.tile([128, 128], BF16)
        identf = per.tile([128, 128], F32)

        with tc.tile_pool(name="prow", bufs=1) as prow:
            # ---- input DMAs ----
            xa_sb = prow.tile([128, NB, CIN + 1], BF16)
            for ch in range(4):
                nc.sync.dma_start(xa_sb[:, bass.ts(ch, 8), :],
                                  xa_d[:, bass.ts(ch, 8), :])
            wq_r = prow.tile([CIN, 128], F32R)
            wk_r = prow.tile([CIN, 128], F32R)
            wqf = prow.tile([CIN, 128], F32)
            wkf = prow.tile([CIN, 128], F32)
            wv_r = prow.tile([CIN, DH], F32R)
            nc.sync.dma_start(wq_r[:], wq_d[:])
            nc.sync.dma_start(wk_r[:], wk_d[:])
            nc.sync.dma_start(wqf[:], wqf_d[:])
            nc.sync.dma_start(wkf[:], wkf_d[:])
            nc.sync.dma_start(wv_r[:], wv_d[:])
            nc.sync.dma_start(wo_r[0:DH + 1, :], wo_d[:])
            nc.sync.dma_start(wo_r[64:97, :], wo_d[:])
            # chunked xT so projections can start on the first quarter
            xTr = prow.tile([CIN, S], F32R)
            for ch in range(4):
                nc.sync.dma_start(xTr[:, bass.ts(ch, 1024)],
                                  xT_d[:, bass.ts(ch, 1024)])

            # ---- ACT table warm-up: force natural_log_exp set load at t=0
            warmt = prow.tile([128, 2], F32)
            nc.vector.memset(warmt[:, 0:1], 1.0)
            nc.scalar.activation(warmt[:, 1:2], warmt[:, 0:1], AF.Ln)
            nc.scalar.activation(warmt[:, 1:2], warmt[:, 0:1], AF.Exp)

            # ---- stats via Gram matrix: G = [x|1]^T [x|1]  ([64, 65]) ----
            ones64 = prow.tile([CIN, 1], F32)
            nc.vector.memset(ones64[:], 1.0)
            with tc.tile_pool(name="gp", bufs=1, space="PSUM") as gp:
                psG = gp.tile([CIN, CIN + 1], F32, name="psG")
                for b in range(NB):
                    nc.tensor.matmul(psG[:], xa_sb[:, b, 0:CIN], xa_sb[:, b, :],
                                     start=(b == 0), stop=(b == NB - 1))
                Gsb = prow.tile([CIN, CIN + 1], F32R)
                nc.vector.tensor_copy(Gsb[:], psG[:])

                def stats_for(w_r, w_f, nm):
                    # mu = w^T sx / S ; ex2 = diag(w^T G w) / S
                    # (f32r matmuls need a moving free dim >= 2: use a 2-col
                    # window of Gsb whose second column is sx)
                    psMu = gp.tile([128, 2], F32, name=f"psMu{nm}")
                    nc.tensor.matmul(psMu[:], w_r[:], Gsb[:, CIN - 1:CIN + 1],
                                     start=True, stop=True)
                    psH = gp.tile([CIN, 128], F32, name=f"psH{nm}")
                    nc.tensor.matmul(psH[:], Gsb[:, 0:CIN], w_r[:],
                                     start=True, stop=True)
                    Hs = prow.tile([CIN, 128], F32, name=f"Hs{nm}")
                    nc.vector.tensor_copy(Hs[:], psH[:])
                    prod = prow.tile([CIN, 128], F32, name=f"prod{nm}")
                    nc.vector.tensor_tensor(out=prod[:], in0=Hs[:], in1=w_f[:],
                                            op=ALU.mult)
                    psSq = gp.tile([128, 1], F32, name=f"psSq{nm}")
                    nc.tensor.matmul(psSq[:], prod[:], ones64[:],
                                     start=True, stop=True)
                    mu = prow.tile([128, 1], F32, name=f"mu{nm}")
                    nc.vector.tensor_scalar_mul(mu[:], psMu[:, 1:2], 1.0 / S)
                    ex2 = prow.tile([128, 1], F32, name=f"ex2{nm}")
                    nc.vector.tensor_scalar_mul(ex2[:], psSq[:], 1.0 / S)
                    return mu, ex2

                mu_q, ex2_q = stats_for(wq_r, wqf, "q")
                mu_k, ex2_k = stats_for(wk_r, wkf, "k")

            def finish_stats(mu, ex2, fold, nm):
                musq = prow.tile([128, 1], F32, name=f"musq{nm}")
                nc.vector.tensor_tensor(out=musq[:], in0=mu[:], in1=mu[:],
                                        op=ALU.mult)
                vareps = prow.tile([128, 1], F32, name=f"vareps{nm}")
                nc.vector.tensor_tensor(out=vareps[:], in0=ex2[:], in1=musq[:],
                                        op=ALU.subtract)
                nc.vector.tensor_scalar_add(vareps[:], vareps[:], EPS)
                # rsqrt seed via exp(-0.5 ln v) (stays in natural_log_exp set)
                lnv = prow.tile([128, 1], F32, name=f"lnv{nm}")
                nc.scalar.activation(lnv[:], vareps[:], AF.Ln)
                r0 = prow.tile([128, 1], F32, name=f"r0{nm}")
                nc.scalar.activation(r0[:], lnv[:], AF.Exp, scale=-0.5)
                # one Newton step: r1 = r0 * (1.5 - 0.5 v r0^2)
                r0sq = prow.tile([128, 1], F32, name=f"r0sq{nm}")
                nc.vector.tensor_tensor(out=r0sq[:], in0=r0[:], in1=r0[:],
                                        op=ALU.mult)
                h = prow.tile([128, 1], F32, name=f"h{nm}")
                nc.vector.tensor_tensor(out=h[:], in0=r0sq[:], in1=vareps[:],
                                        op=ALU.mult)
                w = prow.tile([128, 1], F32, name=f"w{nm}")
                nc.vector.tensor_scalar(out=w[:], in0=h[:], scalar1=-0.5,
                                        scalar2=1.5, op0=ALU.mult, op1=ALU.add)
                rstd = prow.tile([128, 1], F32, name=f"rstd{nm}")
                nc.vector.tensor_tensor(out=rstd[:], in0=r0[:], in1=w[:],
                                        op=ALU.mult)
                if fold != 1.0:
                    nc.vector.tensor_scalar_mul(rstd[:], rstd[:], fold)
                negb = prow.tile([128, 1], F32, name=f"negb{nm}")
                nc.vector.tensor_tensor(out=negb[:], in0=mu[:], in1=rstd[:],
                                        op=ALU.mult)
                nc.vector.tensor_scalar_mul(negb[:], negb[:], -1.0)
                return rstd, negb

            rstd_q, negb_q = finish_stats(mu_q, ex2_q, SCALE, "q")
            rstd_k, negb_k = finish_stats(mu_k, ex2_k, 1.0, "k")

            # ---- projections; drain PSUM straight to standardized bf16 ----
            qhat32 = prow.tile([DH, S], F32)   # 16*qhat rows 0:32, f32
            khat32 = prow.tile([DH, S], F32)   # khat rows 0:32, f32
            vbf = prow.tile([DH, S], BF16)

            with tc.tile_pool(name="props", bufs=2, space="PSUM") as props:
                pp_q = {}
                pp_k = {}
                for half in range(2):
                    # q: ACT drains (qA band + f32 hat rows 0:32)
                    ppq = props.tile([128, 4, 512], F32, name=f"ppq{half}",
                                     tag="pp")
                    for n in range(4):
                        sl = bass.ds(2048 * half + 512 * n, 512)
                        nc.tensor.matmul(ppq[:, n, :], wq_r[:], xTr[:, sl],
                                         start=True, stop=True)
                    nc.scalar.activation(qA[:, bass.ts(half, 2048)], ppq[:],
                                         AF.Identity, bias=negb_q[:],
                                         scale=rstd_q[:])
                    pp_q[half] = ppq
                    # k: DVE drains
                    ppk = props.tile([128, 4, 512], F32, name=f"ppk{half}",
                                     tag="pp")
                    for n in range(4):
                        sl = bass.ds(2048 * half + 512 * n, 512)
                        nc.tensor.matmul(ppk[:, n, :], wk_r[:], xTr[:, sl],
                                         start=True, stop=True)
                    nc.vector.tensor_scalar(out=kA[:, bass.ts(half, 2048)],
                                            in0=ppk[:], scalar1=mu_k[:],
                                            scalar2=rstd_k[:],
                                            op0=ALU.subtract, op1=ALU.mult)
                    pp_k[half] = ppk
                    # deferred f32-hat drains for the hi/lo residuals
                    nc.vector.tensor_scalar(out=qhat32[:, bass.ts(half, 2048)],
                                            in0=ppq[0:DH, :, :],
                                            scalar1=mu_q[0:DH, :],
                                            scalar2=rstd_q[0:DH, :],
                                            op0=ALU.subtract, op1=ALU.mult)
                    nc.scalar.activation(khat32[:, bass.ts(half, 2048)],
                                         ppk[0:DH, :, :], AF.Identity,
                                         bias=negb_k[0:DH, :],
                                         scale=rstd_k[0:DH, :])
                # v projection -> bf16 v (rows 0:32)
                for half in range(2):
                    pv = props.tile([128, 4, 512], F32, name=f"pv{half}",
                                    tag="pp")
                    for n in range(4):
                        nc.tensor.matmul(pv[0:DH, n, :], wv_r[:],
                                         xTr[:, bass.ds(2048 * half + 512 * n,
                                                        512)],
                                         start=True, stop=True)
                    if half == 0:
                        nc.scalar.copy(vbf[:, bass.ts(half, 2048)],
                                       pv[0:DH, :, :])
                    else:
                        nc.vector.tensor_copy(vbf[:, bass.ts(half, 2048)],
                                              pv[0:DH, :, :])

                # ---- vaug: PE transposes of bf16 v -> [j, d|1] blocks ----
                from concourse.masks import make_identity
                make_identity(nc, identb[:])
                make_identity(nc, identf[:])
                nc.gpsimd.memset(vaug[:], 1.0)
                for g in range(8):
                    pt4 = props.tile([128, 4, 512], BF16, name=f"pvt{g}",
                                     tag="pp")
                    for t in range(4):
                        jb = 4 * g + t
                        nc.tensor.transpose(pt4[:, t, 0:DH],
                                            vbf[:, bass.ts(jb, 128)],
                                            identb[0:DH, 0:DH])
                    nc.vector.tensor_copy(vaug[:, 4 * g:4 * g + 4, 0:DH],
                                          pt4[:, :, 0:DH])

            # ---- hi/lo pair tiles ----
            # kP = [khi; khi; klo; -1]; qPc[c] = [qhi; qlo; qhi; mhat]
            nc.sync.dma_start(kP[0:DH, :], kA[0:DH, :])
            nc.sync.dma_start(kP[DH:2 * DH, :], kA[DH:2 * DH, :])
            qlo_t = prow.tile([DH, S], BF16)
            klo_t = prow.tile([DH, S], BF16)
            nc.gpsimd.tensor_tensor(out=qlo_t[:], in0=qhat32[:],
                                    in1=qA[0:DH, :], op=ALU.subtract)
            nc.gpsimd.tensor_tensor(out=klo_t[:], in0=khat32[:],
                                    in1=kA[0:DH, :], op=ALU.subtract)
            nc.sync.dma_start(kP[2 * DH:3 * DH, :], klo_t[:])
            for c in range(NCH):
                cs = bass.ts(c, CHUNK)
                nc.sync.dma_start(qPc[c][0:DH, :], qA[0:DH, cs])
                nc.sync.dma_start(qPc[c][2 * DH:3 * DH, :], qA[2 * DH:3 * DH, cs])
                nc.sync.dma_start(qPc[c][DH:2 * DH, :], qlo_t[:, cs])
            # kP row 96 = -1 via tiny memset + reshape DMA
            nc.gpsimd.memset(neg1[:], -1.0)
            nc.sync.dma_start(kP[96:97, :], neg1[:])

            # ---- prologue-peel: pass A for chunk 0 with a 4-deep PSUM ring;
            # quarters 0,1 exact max on DVE; 2,3 lse bound on ACT ----
            if STAGE >= 2:
                l8all = prow.tile([128, 8], F32, name="l8all")
                m01all = prow.tile([128, 8], F32, name="m01all")
                bias25 = prow.tile([128, 1], F32, name="bias25")
                nc.vector.memset(bias25[:], -25.0)
                with tc.tile_pool(name="pe4", bufs=4, space="PSUM") as pe4, \
                     tc.tile_pool(name="jkp", bufs=2) as jk_pool, \
                     tc.tile_pool(name="mpp", bufs=3) as mp_pool:
                    for blk in range(8):
                        mp2 = mp_pool.tile([128, 2], F32, name=f"mpl{blk}",
                                           tag="mpeel")
                        l8q = mp_pool.tile([128, 2], F32, name=f"l8q{blk}",
                                           tag="l8q")
                        for q in range(4):
                            pa = pe4.tile([128, 1024], F32,
                                          name=f"pa{blk}_{q}", tag="pe4")
                            for r in range(2):
                                nc.tensor.matmul(
                                    pa[:, bass.ts(r, 512)],
                                    qA[bass.ts(r, 32), bass.ts(blk, 128)],
                                    kA[bass.ts(r, 32),
                                       bass.ds(1024 * q + 512 * r, 512)],
                                    start=True, stop=True,
                                    tile_position=(32 * r, 0),
                                )
                            if q < 2:
                                nc.vector.reduce_max(mp2[:, q:q + 1], pa[:],
                                                     axis=mybir.AxisListType.X)
                            else:
                                ju = jk_pool.tile([128, 1024], BF16,
                                                  name=f"ju{blk}_{q}", tag="ju")
                                nc.scalar.activation(ju[:], pa[:], AF.Exp,
                                                     scale=0.0625,
                                                     bias=bias25[:],
                                                     accum_out=l8q[:, q - 2:q - 1])
                        nc.vector.reduce_max(m01all[:, blk:blk + 1], mp2[:],
                                             axis=mybir.AxisListType.X)
                        nc.vector.tensor_tensor(out=l8all[:, blk:blk + 1],
                                                in0=l8q[:, 0:1], in1=l8q[:, 1:2],
                                                op=ALU.add)
                lnt = prow.tile([128, 8], F32, name="lnt")
                nc.scalar.activation(lnt[:], l8all[:], AF.Ln)
                mlse = prow.tile([128, 8], F32, name="mlse")
                # m = 16*(ln l8' + 25) - 40 = 16*ln l8' + 360
                nc.vector.tensor_scalar(out=mlse[:], in0=lnt[:], scalar1=16.0,
                                        scalar2=360.0, op0=ALU.mult, op1=ALU.add)
                nc.vector.tensor_tensor(out=mcolT[0][:], in0=m01all[:],
                                        in1=mlse[:], op=ALU.max)

        # ================= main loop =================
        with tc.tile_pool(name="uni", bufs=3, space="PSUM") as uni_pool, \
             tc.tile_pool(name="psAV", bufs=1, space="PSUM") as psAV_pool, \
             tc.tile_pool(name="mpp2", bufs=3) as mp_pool, \
             tc.tile_pool(name="ptp", bufs=4) as pt_pool, \
             tc.tile_pool(name="epp", bufs=2) as ep_pool:

            if STAGE <= 2:
                zout = ep_pool.tile([CIN, S], F32, name="zout", tag="zout")
                nc.vector.memset(zout[:], 0.0)
                nc.sync.dma_start(out_d[:], zout[:])

            mparts_t = {}

            def emit_passA_quarter(blk, q):
                if q == 0:
                    mparts_t[blk] = mp_pool.tile([128, 4], F32, name=f"mp{blk}",
                                                 tag="mparts")
                mp = mparts_t[blk]
                pa = uni_pool.tile([128, 1024], F32, name=f"pa{blk}_{q}",
                                   tag="uni")
                for r in range(2):
                    nc.tensor.matmul(
                        pa[:, bass.ts(r, 512)],
                        qA[bass.ts(r, 32), bass.ts(blk, 128)],
                        kA[bass.ts(r, 32), bass.ds(1024 * q + 512 * r, 512)],
                        start=True, stop=True,
                        tile_position=(32 * r, 0),
                    )
                nc.vector.reduce_max(mp[:, q:q + 1], pa[:],
                                     axis=mybir.AxisListType.X)
                if q == 3:
                    mparts_t.pop(blk)
                    nc.vector.reduce_max(
                        mcolT[blk // 8][:, (blk % 8):(blk % 8) + 1], mp[:],
                        axis=mybir.AxisListType.X)

            def emit_mhat(c):
                # 8 max columns -> PE transpose -> bf16 row -> reshape DMA into
                # row 96 of qPc[c]
                psm = uni_pool.tile([128, 1024], F32, name=f"psm{c}", tag="uni")
                nc.tensor.transpose(psm[0:8, 0:128], mcolT[c][:], identf[:])
                m8 = ep_pool.tile([8, 128], BF16, name=f"m8_{c}", tag="m8")
                nc.vector.tensor_copy(m8[:], psm[0:8, 0:128])
                # explicit 3D dst AP pins descriptor order (block-major)
                nc.sync.dma_start(
                    qPc[c][96:97, :].rearrange("a (b c) -> a b c", b=8), m8[:])

            avs = {}
            pts = {}

            def emit_AV_pair(c, jb0):
                # col-tiled concurrent pair: even jb -> partitions 0:33 at
                # tile_position (0,0); odd jb -> partitions 64:97 at (0,64)
                avh = avs[c]
                pte = pts.pop(jb0)
                pto = pts.pop(jb0 + 1)
                for hf in range(2):
                    nc.tensor.matmul(avh[0:DH + 1, bass.ts(hf, 512)],
                                     vaug[:, jb0, :], pte[:, bass.ts(hf, 512)],
                                     start=(jb0 == 0), stop=(jb0 == NB - 2),
                                     tile_position=(0, 0))
                    nc.tensor.matmul(avh[64:64 + DH + 1, bass.ts(hf, 512)],
                                     vaug[:, jb0 + 1, :],
                                     pto[:, bass.ts(hf, 512)],
                                     start=(jb0 == 0), stop=(jb0 == NB - 2),
                                     tile_position=(0, 64))

            def epilogue_steps(c):
                # chunk epilogue split into closures, one per early iteration
                # of the next chunk, to spread PSUM-ring + DVE pressure.
                # Emits the UNNORMALIZED projection wo^T @ [av; l] plus the
                # denominator row; the host divides during unshard.
                avh = avs.pop(c)
                avsb = ep_pool.tile([DH + 1, CHUNK], F32R, name=f"avsb{c}",
                                    tag="avsb", bufs=4)

                def s0():
                    # combine the two col-tiled AV halves (DVE can read only
                    # one PSUM operand per instruction)
                    nc.vector.tensor_copy(avsb[:], avh[0:DH + 1, :])
                    nc.vector.tensor_tensor(out=avsb[:], in0=avsb[:],
                                            in1=avh[64:64 + DH + 1, :],
                                            op=ALU.add)
                    nc.sync.dma_start(l_d[:, bass.ts(c, CHUNK)], avsb[32:33, :])

                def seg_step(seg):
                    def s():
                        sg = bass.ts(seg, 512)
                        psY = uni_pool.tile([128, 1024], F32, name=f"psY{c}_{seg}",
                                            tag="uni")
                        nc.tensor.matmul(psY[0:CIN, 0:512], wo_r[0:DH + 1, :],
                                         avsb[:, sg], start=True, stop=True)
                        ysb = ep_pool.tile([CIN, 512], F32, name=f"ysb{c}_{seg}",
                                           tag="ysb")
                        nc.vector.tensor_copy(ysb[:], psY[0:CIN, 0:512])
                        nc.sync.dma_start(out_d[:, bass.ds(CHUNK * c + 512 * seg,
                                                           512)], ysb[:])
                    return s

                return [s0, seg_step(0), seg_step(1)]

            if STAGE == 2:
                zout = ep_pool.tile([CIN, S], F32, name="zout", tag="zout")
                nc.vector.memset(zout[:], 0.0)
                nc.sync.dma_start(out_d[:], zout[:])
            pending = []
            for c in range(NCH if STAGE >= 3 else 0):
                emit_mhat(c)
                if c > 0 and STAGE >= 4:
                    steps = epilogue_steps(c - 1)
                    # step 0 (avh halves -> SBUF add) must precede the
                    # reallocation of the single-buffer AV accumulator below
                    steps[0]()
                    pending = steps[1:]
                elif c > 0:
                    avs.pop(c - 1)
                avs[c] = psAV_pool.tile([128, CHUNK], F32, name=f"av{c}",
                                        tag="av")
                for jb in range(NB):
                    if c + 1 < NCH:
                        emit_passA_quarter(8 * (c + 1) + jb // 4, jb % 4)
                    psB = uni_pool.tile([128, CHUNK], F32, name=f"psB{c}_{jb}",
                                        tag="uni")
                    for hf in range(2):
                        nc.tensor.matmul(psB[:, bass.ts(hf, 512)],
                                         kP[:, bass.ts(jb, 128)],
                                         qPc[c][:, bass.ts(hf, 512)],
                                         start=True, stop=True)
                    pt = pt_pool.tile([128, CHUNK], BF16, name=f"pt{c}_{jb}",
                                      tag="pt")
                    nc.scalar.activation(pt[:], psB[:], AF.Exp)
                    pts[jb] = pt
                    if jb >= 2 and jb % 2 == 0:
                        emit_AV_pair(c, jb - 2)
                    if pending and jb >= 2:
                        pending.pop(0)()
                emit_AV_pair(c, NB - 2)
            for step in pending:
                step()
            if STAGE >= 4:
                for step in epilogue_steps(NCH - 1):
                    step()
            elif STAGE == 3:
                avs.pop(NCH - 1)
                zout = ep_pool.tile([CIN, S], F32, name="zout", tag="zout")
                nc.vector.memset(zout[:], 0.0)
                nc.sync.dma_start(out_d[:], zout[:])

    nc.compile()
    return nc


def _get_compiled():
    global _compiled
    if _compiled is None:
        _compiled = _build()
    return _compiled


def kernel(input, w_qkv, w_out, b_out):
    import ml_dtypes
    input = np.asarray(input, dtype=np.float32)
    w_qkv = np.asarray(w_qkv, dtype=np.float32)
    w_out = np.asarray(w_out, dtype=np.float32)
    b_out = np.asarray(b_out, dtype=np.float32)
    b, x, y, z, c = input.shape
    assert (b, x, y, z, c) == (2, 16, 16, 16, 64)
    hid = HEADS * DH

    xa_by_batch = []
    for bb in range(b):
        xf = input[bb].reshape(S, CIN)
        aug = np.concatenate([xf, np.ones((S, 1), np.float32)], axis=1)
        xa = np.ascontiguousarray(
            aug.reshape(NB, 128, CIN + 1).transpose(1, 0, 2)
        ).astype(ml_dtypes.bfloat16)
        xa_by_batch.append(xa)

    in_maps = []
    for core in range(8):
        bb, h = divmod(core, HEADS)
        xT = np.ascontiguousarray(input[bb].reshape(S, CIN).T)
        wq = np.tile(w_qkv[:, h * DH:(h + 1) * DH], (1, 4))
        wk = np.tile(w_qkv[:, hid + h * DH: hid + (h + 1) * DH], (1, 4))
        wv = np.ascontiguousarray(w_qkv[:, 2 * hid + h * DH: 2 * hid + (h + 1) * DH])
        wo = np.vstack([w_out[h * DH:(h + 1) * DH, :], b_out[None, :] / HEADS])
        wq = np.ascontiguousarray(wq)
        wk = np.ascontiguousarray(wk)
        in_maps.append({
            "xT": xT,
            "xa": xa_by_batch[bb],
            "wq": wq,
            "wk": wk,
            "wqf": wq.copy(),
            "wkf": wk.copy(),
            "wv": wv,
            "wo": np.ascontiguousarray(wo),
        })

    global _last_in_maps
    _last_in_maps = in_maps
    nc = _get_compiled()
    res = run_bass_kernel_spmd(nc, in_maps, core_ids=list(range(8)))
    out = np.zeros((b, S, CIN), dtype=np.float32)
    for core in range(8):
        bb = core // HEADS
        num = res.results[core]["out"]          # [64, S], unnormalized
        l = res.results[core]["ldenom"][0]      # [S]
        out[bb] += (num / l[None, :]).T
    return out.reshape(b, x, y, z, CIN)


if __name__ == "__main__":
    rng = np.random.default_rng(0)
    inp = rng.standard_normal((2, 16, 16, 16, 64), dtype=np.float32)
    wqkv = rng.standard_normal((64, 384), dtype=np.float32) / 8.0
    wout = rng.standard_normal((128, 64), dtype=np.float32) / np.sqrt(128)
    bout = np.zeros(64, dtype=np.float32)
    o = kernel(inp, wqkv, wout, bout)
    print("kernel output shape:", o.shape)


# revision 22
# speedup vs baseline: 1.0044x; 1.0044x over previous
"""Trainium2 Bass kernel for 3D volume attention (b=2, x=y=z=16, c=64,
heads=4, dim_head=32, qk-standardize over sequence, scale=16).

Sharding: batch*heads = 8 (b,h) pairs -> 8 NeuronCores, one pair per core.
Host pre-transposes x and pre-slices per-head weights; host sums the 4
head-partials per batch (pure unshard-reduce) and reshapes.

Per-core pipeline (s=4096, d=32). v2 rewrite of the two-pass softmax
kernel, tuned from a perfetto trace of v1 (307us):
  prologue: standardization stats computed via tiny PE matmuls on the
            Gram matrix G = [x|1]^T [x|1] (sumsq_d = w_d^T G w_d,
            mean from the ones column) instead of serial ACT Square
            passes; rsqrt via exp(-0.5 ln v) + one Newton step so the
            WHOLE kernel uses only the natural_log_exp ACT table set
            (no table thrash); projections drain PSUM directly to the
            standardized bf16 qA (ACT) / kA (DVE) replicas; f32-hat
            rows 0:32 drained separately for the hi/lo residuals
            (gpsimd subtract); input DMA chunked so the PE starts
            early; bf16 v^T/identity transposes.
  pass A  (S[i,j], 2-way row-tiled concurrent matmul pairs): per-quarter
          DVE reduce_max; chunk 0 peeled with a dedicated 4-deep PSUM
          ring, quarters 0,1 exact max on DVE + quarters 2,3 lse bound
          on ACT (16*ln(sum exp(s/16)) - 40 >= max-40).
  pass B  (S^T[j,i]): K=97 bf16 matmul ([khi;khi;klo;-1] x
          [qhi;qlo;qhi;mhat]) -> ACT exp -> bf16 P^T.
  AV:     2-way COLUMN-TILED concurrent pairs: even j-blocks accumulate
          P^T @ [v|1] into PSUM partitions 0:33 (tile_position (0,0)),
          odd j-blocks into partitions 64:97 (tile_position (0,64));
          the epilogue adds the halves. Halves the AV stream time.
  out:    per chunk: DVE add of the two AV halves -> SBUF, project with
          [w_out_h ; b_out/4], DVE copy, DMA out. Host divides by the
          returned softmax denominator during unshard.

Software pipelining: AV lags exp by two iterations (pairs), pass A for
chunk c+1 interleaved one quarter per pass-B iteration. PSUM: 3x[128,1024]
shared ring + [128,1024] col-tiled AV accumulator (8 banks exactly).
"""
import os
import sys
from contextlib import ExitStack

import numpy as np

_PROBLEM_DIR = os.path.dirname(os.path.abspath(__file__))
if _PROBLEM_DIR not in sys.path:
    sys.path.insert(0, _PROBLEM_DIR)

import concourse.bass as bass
import concourse.tile as tile
from concourse import bacc, mybir
from concourse.bass_utils import run_bass_kernel_spmd

F32 = mybir.dt.float32
F32R = mybir.dt.float32r
BF16 = mybir.dt.bfloat16
AF = mybir.ActivationFunctionType
ALU = mybir.AluOpType

HEADS = 4
DH = 32          # dim head
CIN = 64         # input channels
S = 4096         # sequence (16^3)
SCALE = 16.0
EPS = 1e-5
NB = S // 128    # 32 j blocks
NCH = 4          # i chunks
CHUNK = 1024
KP = 97          # 3*32 pair rows + 1 aug row

_compiled = None
STAGE = int(os.environ.get("STAGE", "4"))  # 1=prologue 2=+peel 3=+main-loop 4=full


def _build():
    nc = bacc.Bacc("TRN2", target_bir_lowering=False, debug=False, num_devices=8)
    xT_d = nc.dram_tensor("xT", [CIN + 1, S], F32R, kind="ExternalInput").ap()
    xa_d = nc.dram_tensor("xa", [128, NB, CIN + 1], BF16, kind="ExternalInput").ap()
    wq_d = nc.dram_tensor("wq", [CIN, 128], F32R, kind="ExternalInput").ap()
    wk_d = nc.dram_tensor("wk", [CIN, 128], F32R, kind="ExternalInput").ap()
    wqf_d = nc.dram_tensor("wqf", [CIN, 128], F32, kind="ExternalInput").ap()
    wkf_d = nc.dram_tensor("wkf", [CIN, 128], F32, kind="ExternalInput").ap()
    wv_d = nc.dram_tensor("wv", [CIN, DH], F32R, kind="ExternalInput").ap()
    wo_d = nc.dram_tensor("wo", [DH + 1, CIN], F32R, kind="ExternalInput").ap()
    out_d = nc.dram_tensor("out", [CIN, S], F32, kind="ExternalOutput").ap()
    # softmax denominator per column; host divides during unshard (standard
    # split-softmax partial combination)
    l_d = nc.dram_tensor("ldenom", [1, S], F32R, kind="ExternalOutput").ap()

    with tile.TileContext(nc) as tc, ExitStack() as ctx:
        per = ctx.enter_context(tc.tile_pool(name="per", bufs=1))

        # ---- persistent SBUF ----
        wo_r = per.tile([97, CIN], F32R)  # wo at rows 0:33 AND 64:97
        qA = per.tile([128, S], BF16)          # 4 replicated bands of qhat*16
        kA = per.tile([128, S], BF16)          # 4 replicated bands of khat
        kP = per.tile([KP, S], BF16)           # [khi; khi; klo; -1]
        vaug = per.tile([128, NB, DH + 1], BF16)   # per j-block [v | 1]
        qPc = [per.tile([KP, CHUNK], BF16, name=f"qPc{c}") for c in range(NCH)]
        mcolT = [per.tile([128, 8], F32, name=f"mcolT{c}") for c in range(NCH)]
        neg1 = per.tile([128, DH], BF16)
        identb = per.tile([128, 128], BF16)
        identf = per.tile([128, 128], F32)

        with tc.tile_pool(name="prow", bufs=1) as prow:
            # ---- input DMAs ----
            xa_sb = prow.tile([128, NB, CIN + 1], BF16)
            nc.sync.dma_start(xa_sb[:], xa_d[:])
            # augmented projection weights: row 64 = -mu (written on device)
            wq_aug = prow.tile([CIN + 1, 128], F32R)
            wk_aug = prow.tile([CIN + 1, 128], F32R)
            wqf = prow.tile([CIN, 128], F32)
            wkf = prow.tile([CIN, 128], F32)
            wv_r = prow.tile([CIN, DH], F32R)
            nc.sync.dma_start(wq_aug[0:CIN, :], wq_d[:])
            nc.sync.dma_start(wk_aug[0:CIN, :], wk_d[:])
            nc.sync.dma_start(wqf[:], wqf_d[:])
            nc.sync.dma_start(wkf[:], wkf_d[:])
            nc.sync.dma_start(wv_r[:], wv_d[:])
            nc.sync.dma_start(wo_r[0:DH + 1, :], wo_d[:])
            nc.sync.dma_start(wo_r[64:97, :], wo_d[:])
            xTr = prow.tile([CIN + 1, S], F32R)  # row 64 = ones (host-side)
            nc.sync.dma_start(xTr[:], xT_d[:])

            from concourse.masks import make_identity
            make_identity(nc, identb[:])
            make_identity(nc, identf[:])

            # ---- stats via Gram matrix: G = [x|1]^T [x|1]  ([64, 65]) ----
            ones64 = prow.tile([CIN, 1], F32)
            nc.vector.memset(ones64[:], 1.0)
            mu2 = prow.tile([128, 2], F32)    # col 0 = q, col 1 = k
            ex22 = prow.tile([128, 2], F32)
            with tc.tile_pool(name="gp", bufs=1, space="PSUM") as gp:
                psG = gp.tile([CIN, CIN + 1], F32, name="psG")
                for b in range(NB):
                    nc.tensor.matmul(psG[:], xa_sb[:, b, 0:CIN], xa_sb[:, b, :],
                                     start=(b == 0), stop=(b == NB - 1))
                Gsb = prow.tile([CIN, CIN + 1], F32R)
                nc.vector.tensor_copy(Gsb[:], psG[:])

                def stats_for(w_r, w_f, col, nm):
                    # mu = w^T sx / S ; ex2 = diag(w^T G w) / S
                    # (f32r matmuls need a moving free dim >= 2: use a 2-col
                    # window of Gsb whose second column is sx)
                    psMu = gp.tile([128, 2], F32, name=f"psMu{nm}")
                    nc.tensor.matmul(psMu[:], w_r[:], Gsb[:, CIN - 1:CIN + 1],
                                     start=True, stop=True)
                    psH = gp.tile([CIN, 128], F32, name=f"psH{nm}")
                    nc.tensor.matmul(psH[:], Gsb[:, 0:CIN], w_r[:],
                                     start=True, stop=True)
                    Hs = prow.tile([CIN, 128], F32, name=f"Hs{nm}")
                    nc.vector.tensor_copy(Hs[:], psH[:])
                    prod = prow.tile([CIN, 128], F32, name=f"prod{nm}")
                    nc.vector.tensor_tensor(out=prod[:], in0=Hs[:], in1=w_f[:],
                                            op=ALU.mult)
                    psSq = gp.tile([128, 1], F32, name=f"psSq{nm}")
                    nc.tensor.matmul(psSq[:], prod[:], ones64[:],
                                     start=True, stop=True)
                    nc.vector.tensor_scalar_mul(mu2[:, col:col + 1],
                                                psMu[:, 1:2], 1.0 / S)
                    nc.vector.tensor_scalar_mul(ex22[:, col:col + 1],
                                                psSq[:], 1.0 / S)

                stats_for(wq_aug[0:CIN, :], wqf, 0, "q")
                stats_for(wk_aug[0:CIN, :], wkf, 1, "k")

                # -mu rows for the mean-subtracting projections (two separate
                # transposes so both rows land at partition 0 -- DVE reads
                # need a 32-aligned partition base)
                psmuT = gp.tile([1, 256], F32, name="psmuT")
                nc.tensor.transpose(psmuT[0:1, 0:128], mu2[:, 0:1], identf[:])
                nc.tensor.transpose(psmuT[0:1, 128:256], mu2[:, 1:2],
                                    identf[:])
                nc.vector.tensor_scalar_mul(wq_aug[CIN:CIN + 1, :],
                                            psmuT[0:1, 0:128], -1.0)
                nc.vector.tensor_scalar_mul(wk_aug[CIN:CIN + 1, :],
                                            psmuT[0:1, 128:256], -1.0)

            # rstd for q (x16 fold) and k, batched so ln/exp each cost one
            # ACT table-set switch; one Newton polish step
            musq2 = prow.tile([128, 2], F32)
            nc.vector.tensor_tensor(out=musq2[:], in0=mu2[:], in1=mu2[:],
                                    op=ALU.mult)
            vareps2 = prow.tile([128, 2], F32)
            nc.vector.tensor_tensor(out=vareps2[:], in0=ex22[:], in1=musq2[:],
                                    op=ALU.subtract)
            nc.vector.tensor_scalar_add(vareps2[:], vareps2[:], EPS)
            lnv2 = prow.tile([128, 2], F32)
            nc.scalar.activation(lnv2[:], vareps2[:], AF.Ln)
            r02 = prow.tile([128, 2], F32)
            nc.scalar.activation(r02[:], lnv2[:], AF.Exp, scale=-0.5)
            r0sq2 = prow.tile([128, 2], F32)
            nc.vector.tensor_tensor(out=r0sq2[:], in0=r02[:], in1=r02[:],
                                    op=ALU.mult)
            h2 = prow.tile([128, 2], F32)
            nc.vector.tensor_tensor(out=h2[:], in0=r0sq2[:], in1=vareps2[:],
                                    op=ALU.mult)
            w2 = prow.tile([128, 2], F32)
            nc.vector.tensor_scalar(out=w2[:], in0=h2[:], scalar1=-0.5,
                                    scalar2=1.5, op0=ALU.mult, op1=ALU.add)
            fold2 = prow.tile([128, 2], F32)
            nc.vector.memset(fold2[:, 0:1], SCALE)
            nc.vector.memset(fold2[:, 1:2], 1.0)
            rstd2r = prow.tile([128, 2], F32)
            nc.vector.tensor_tensor(out=rstd2r[:], in0=r02[:], in1=w2[:],
                                    op=ALU.mult)
            rstd2 = prow.tile([128, 2], F32)
            nc.vector.tensor_tensor(out=rstd2[:], in0=rstd2r[:], in1=fold2[:],
                                    op=ALU.mult)

            # ---- projections (mean already subtracted via the -mu row);
            # drains go straight to standardized bf16 + bf16 lo-residuals ----
            qlo_t = prow.tile([DH, S], BF16)
            klo_t = prow.tile([DH, S], BF16)
            vbf = prow.tile([DH, S], BF16)

            with tc.tile_pool(name="props", bufs=2, space="PSUM") as props:
                for half in range(2):
                    ppq = props.tile([128, 4, 512], F32, name=f"ppq{half}",
                                     tag="pp")
                    for n in range(4):
                        sl = bass.ds(2048 * half + 512 * n, 512)
                        nc.tensor.matmul(ppq[:, n, :], wq_aug[:], xTr[:, sl],
                                         start=True, stop=True)
                    nc.scalar.activation(qA[:, bass.ts(half, 2048)], ppq[:],
                                         AF.Identity, scale=rstd2[:, 0:1])
                    # lo residual: (psum*rstd) - qA, fused on DVE
                    nc.vector.scalar_tensor_tensor(
                        out=qlo_t[:, bass.ts(half, 2048)],
                        in0=ppq[0:DH, :, :], scalar=rstd2[0:DH, 0:1],
                        in1=qA[0:DH, bass.ts(half, 2048)],
                        op0=ALU.mult, op1=ALU.subtract)
                    ppk = props.tile([128, 4, 512], F32, name=f"ppk{half}",
                                     tag="pp")
                    for n in range(4):
                        sl = bass.ds(2048 * half + 512 * n, 512)
                        nc.tensor.matmul(ppk[:, n, :], wk_aug[:], xTr[:, sl],
                                         start=True, stop=True)
                    nc.scalar.activation(kA[:, bass.ts(half, 2048)], ppk[:],
                                         AF.Identity, scale=rstd2[:, 1:2])
                    nc.vector.scalar_tensor_tensor(
                        out=klo_t[:, bass.ts(half, 2048)],
                        in0=ppk[0:DH, :, :], scalar=rstd2[0:DH, 1:2],
                        in1=kA[0:DH, bass.ts(half, 2048)],
                        op0=ALU.mult, op1=ALU.subtract)
                # v projection -> bf16 v (rows 0:32)
                for half in range(2):
                    pv = props.tile([128, 4, 512], F32, name=f"pv{half}",
                                    tag="pp")
                    for n in range(4):
                        nc.tensor.matmul(pv[0:DH, n, :], wv_r[:],
                                         xTr[0:CIN,
                                             bass.ds(2048 * half + 512 * n,
                                                     512)],
                                         start=True, stop=True)
                    if half == 0:
                        nc.scalar.copy(vbf[:, bass.ts(half, 2048)],
                                       pv[0:DH, :, :])
                    else:
                        nc.vector.tensor_copy(vbf[:, bass.ts(half, 2048)],
                                              pv[0:DH, :, :])

                # ---- vaug: PE transposes of bf16 v -> [j, d|1] blocks ----
                nc.gpsimd.memset(vaug[:], 1.0)
                for g in range(8):
                    pt4 = props.tile([128, 4, 512], BF16, name=f"pvt{g}",
                                     tag="pp")
                    for t in range(4):
                        jb = 4 * g + t
                        nc.tensor.transpose(pt4[:, t, 0:DH],
                                            vbf[:, bass.ts(jb, 128)],
                                            identb[0:DH, 0:DH])
                    nc.vector.tensor_copy(vaug[:, 4 * g:4 * g + 4, 0:DH],
                                          pt4[:, :, 0:DH])

            # ---- hi/lo pair tiles ----
            # kP = [khi; khi; klo; -1]; qPc[c] = [qhi; qlo; qhi; mhat]
            nc.sync.dma_start(kP[0:DH, :], kA[0:DH, :])
            nc.sync.dma_start(kP[DH:2 * DH, :], kA[DH:2 * DH, :])
            nc.sync.dma_start(kP[2 * DH:3 * DH, :], klo_t[:])
            for c in range(NCH):
                cs = bass.ts(c, CHUNK)
                nc.sync.dma_start(qPc[c][0:DH, :], qA[0:DH, cs])
                nc.sync.dma_start(qPc[c][2 * DH:3 * DH, :], qA[2 * DH:3 * DH, cs])
                nc.sync.dma_start(qPc[c][DH:2 * DH, :], qlo_t[:, cs])
            # kP row 96 = -1 via tiny memset + reshape DMA
            nc.gpsimd.memset(neg1[:], -1.0)
            nc.sync.dma_start(kP[96:97, :], neg1[:])

            # ---- prologue-peel: pass A for chunk 0 with a 4-deep PSUM ring;
            # quarters 0,1 exact max on DVE; 2,3 lse bound on ACT ----
            if STAGE >= 2:
                l8all = prow.tile([128, 8], F32, name="l8all")
                m01all = prow.tile([128, 8], F32, name="m01all")
                bias25 = prow.tile([128, 1], F32, name="bias25")
                nc.vector.memset(bias25[:], -25.0)
                with tc.tile_pool(name="pe4", bufs=4, space="PSUM") as pe4, \
                     tc.tile_pool(name="jkp", bufs=2) as jk_pool, \
                     tc.tile_pool(name="mpp", bufs=3) as mp_pool:
                    for blk in range(8):
                        mp2 = mp_pool.tile([128, 2], F32, name=f"mpl{blk}",
                                           tag="mpeel")
                        l8q = mp_pool.tile([128, 2], F32, name=f"l8q{blk}",
                                           tag="l8q")
                        for q in range(4):
                            pa = pe4.tile([128, 1024], F32,
                                          name=f"pa{blk}_{q}", tag="pe4")
                            for r in range(2):
                                nc.tensor.matmul(
                                    pa[:, bass.ts(r, 512)],
                                    qA[bass.ts(r, 32), bass.ts(blk, 128)],
                                    kA[bass.ts(r, 32),
                                       bass.ds(1024 * q + 512 * r, 512)],
                                    start=True, stop=True,
                                    tile_position=(32 * r, 0),
                                )
                            if q < 2:
                                nc.vector.reduce_max(mp2[:, q:q + 1], pa[:],
                                                     axis=mybir.AxisListType.X)
                            else:
                                ju = jk_pool.tile([128, 1024], BF16,
                                                  name=f"ju{blk}_{q}", tag="ju")
                                nc.scalar.activation(ju[:], pa[:], AF.Exp,
                                                     scale=0.0625,
                                                     bias=bias25[:],
                                                     accum_out=l8q[:, q - 2:q - 1])
                        nc.vector.reduce_max(m01all[:, blk:blk + 1], mp2[:],
                                             axis=mybir.AxisListType.X)
                        nc.vector.tensor_tensor(out=l8all[:, blk:blk + 1],
                                                in0=l8q[:, 0:1], in1=l8q[:, 1:2],
                                                op=ALU.add)
                lnt = prow.tile([128, 8], F32, name="lnt")
                nc.scalar.activation(lnt[:], l8all[:], AF.Ln)
                mlse = prow.tile([128, 8], F32, name="mlse")
                # m = 16*(ln l8' + 25) - 40 = 16*ln l8' + 360
                nc.vector.tensor_scalar(out=mlse[:], in0=lnt[:], scalar1=16.0,
                                        scalar2=360.0, op0=ALU.mult, op1=ALU.add)
                nc.vector.tensor_tensor(out=mcolT[0][:], in0=m01all[:],
                                        in1=mlse[:], op=ALU.max)

        # ================= main loop =================
        with tc.tile_pool(name="uni", bufs=3, space="PSUM") as uni_pool, \
             tc.tile_pool(name="psAV", bufs=1, space="PSUM") as psAV_pool, \
             tc.tile_pool(name="mpp2", bufs=3) as mp_pool, \
             tc.tile_pool(name="ptp", bufs=4) as pt_pool, \
             tc.tile_pool(name="epp", bufs=2) as ep_pool:

            if STAGE <= 2:
                zout = ep_pool.tile([CIN, S], F32, name="zout", tag="zout")
                nc.vector.memset(zout[:], 0.0)
                nc.sync.dma_start(out_d[:], zout[:])

            mparts_t = {}

            def emit_passA_quarter(blk, q):
                if q == 0:
                    mparts_t[blk] = mp_pool.tile([128, 4], F32, name=f"mp{blk}",
                                                 tag="mparts")
                mp = mparts_t[blk]
                pa = uni_pool.tile([128, 1024], F32, name=f"pa{blk}_{q}",
                                   tag="uni")
                for r in range(2):
                    nc.tensor.matmul(
                        pa[:, bass.ts(r, 512)],
                        qA[bass.ts(r, 32), bass.ts(blk, 128)],
                        kA[bass.ts(r, 32), bass.ds(1024 * q + 512 * r, 512)],
                        start=True, stop=True,
                        tile_position=(32 * r, 0),
                    )
                nc.vector.reduce_max(mp[:, q:q + 1], pa[:],
                                     axis=mybir.AxisListType.X)
                if q == 3:
                    mparts_t.pop(blk)
                    nc.vector.reduce_max(
                        mcolT[blk // 8][:, (blk % 8):(blk % 8) + 1], mp[:],
                        axis=mybir.AxisListType.X)

            def emit_mhat(c):
                # 8 max columns -> PE transpose -> bf16 row -> reshape DMA into
                # row 96 of qPc[c]
                psm = uni_pool.tile([128, 1024], F32, name=f"psm{c}", tag="uni")
                nc.tensor.transpose(psm[0:8, 0:128], mcolT[c][:], identf[:])
                m8 = ep_pool.tile([8, 128], BF16, name=f"m8_{c}", tag="m8")
                nc.vector.tensor_copy(m8[:], psm[0:8, 0:128])
                # explicit 3D dst AP pins descriptor order (block-major)
                nc.sync.dma_start(
                    qPc[c][96:97, :].rearrange("a (b c) -> a b c", b=8), m8[:])

            avs = {}
            pts = {}

            def emit_AV_pair(c, jb0):
                # col-tiled concurrent pairs: even jb -> partitions 0:33 at
                # tile_position (0,0); odd jb -> partitions 64:97 at (0,64).
                # Adjacent matmuls must hit DIFFERENT PSUM banks to stream
                # concurrently, so interleave as (E-h0 | O-h1), (O-h0 | E-h1).
                avh = avs[c]
                pte = pts.pop(jb0)
                pto = pts.pop(jb0 + 1)
                st = (jb0 == 0)
                sp = (jb0 == NB - 2)
                nc.tensor.matmul(avh[0:DH + 1, 0:512], vaug[:, jb0, :],
                                 pte[:, 0:512], start=st, stop=sp,
                                 tile_position=(0, 0))
                nc.tensor.matmul(avh[64:64 + DH + 1, 512:1024],
                                 vaug[:, jb0 + 1, :], pto[:, 512:1024],
                                 start=st, stop=sp, tile_position=(0, 64))
                nc.tensor.matmul(avh[64:64 + DH + 1, 0:512],
                                 vaug[:, jb0 + 1, :], pto[:, 0:512],
                                 start=st, stop=sp, tile_position=(0, 64))
                nc.tensor.matmul(avh[0:DH + 1, 512:1024], vaug[:, jb0, :],
                                 pte[:, 512:1024], start=st, stop=sp,
                                 tile_position=(0, 0))

            def epilogue_steps(c):
                # chunk epilogue split into closures, one per early iteration
                # of the next chunk, to spread PSUM-ring + DVE pressure.
                # Emits the UNNORMALIZED projection wo^T @ [av; l] plus the
                # denominator row; the host divides during unshard.
                avh = avs.pop(c)
                avsb = ep_pool.tile([DH + 1, CHUNK], F32R, name=f"avsb{c}",
                                    tag="avsb", bufs=4)

                def s0():
                    # combine the two col-tiled AV halves (DVE can read only
                    # one PSUM operand per instruction)
                    nc.vector.tensor_copy(avsb[:], avh[0:DH + 1, :])
                    nc.vector.tensor_tensor(out=avsb[:], in0=avsb[:],
                                            in1=avh[64:64 + DH + 1, :],
                                            op=ALU.add)
                    nc.sync.dma_start(l_d[:, bass.ts(c, CHUNK)], avsb[32:33, :])

                def seg_step(seg):
                    def s():
                        sg = bass.ts(seg, 512)
                        psY = uni_pool.tile([128, 1024], F32, name=f"psY{c}_{seg}",
                                            tag="uni")
                        nc.tensor.matmul(psY[0:CIN, 0:512], wo_r[0:DH + 1, :],
                                         avsb[:, sg], start=True, stop=True)
                        ysb = ep_pool.tile([CIN, 512], F32, name=f"ysb{c}_{seg}",
                                           tag="ysb")
                        nc.vector.tensor_copy(ysb[:], psY[0:CIN, 0:512])
                        nc.sync.dma_start(out_d[:, bass.ds(CHUNK * c + 512 * seg,
                                                           512)], ysb[:])
                    return s

                return [s0, seg_step(0), seg_step(1)]

            if STAGE == 2:
                zout = ep_pool.tile([CIN, S], F32, name="zout", tag="zout")
                nc.vector.memset(zout[:], 0.0)
                nc.sync.dma_start(out_d[:], zout[:])
            pending = []
            for c in range(NCH if STAGE >= 3 else 0):
                if c == 0:
                    emit_mhat(0)
                if c > 0 and STAGE >= 4:
                    steps = epilogue_steps(c - 1)
                    # step 0 (avh halves -> SBUF add) must precede the
                    # reallocation of the single-buffer AV accumulator below
                    steps[0]()
                    pending = steps[1:]
                elif c > 0:
                    avs.pop(c - 1)
                avs[c] = psAV_pool.tile([128, CHUNK], F32, name=f"av{c}",
                                        tag="av")
                for jb in range(NB):
                    if c + 1 < NCH:
                        emit_passA_quarter(8 * (c + 1) + jb // 4, jb % 4)
                        if jb == NB - 1:
                            # next chunk's mhat row, well before its pass B
                            emit_mhat(c + 1)
                    psB = uni_pool.tile([128, CHUNK], F32, name=f"psB{c}_{jb}",
                                        tag="uni")
                    for hf in range(2):
                        nc.tensor.matmul(psB[:, bass.ts(hf, 512)],
                                         kP[:, bass.ts(jb, 128)],
                                         qPc[c][:, bass.ts(hf, 512)],
                                         start=True, stop=True)
                    pt = pt_pool.tile([128, CHUNK], BF16, name=f"pt{c}_{jb}",
                                      tag="pt")
                    nc.scalar.activation(pt[:], psB[:], AF.Exp)
                    pts[jb] = pt
                    if jb >= 2 and jb % 2 == 0:
                        emit_AV_pair(c, jb - 2)
                    if pending and jb >= 2:
                        pending.pop(0)()
                emit_AV_pair(c, NB - 2)
            for step in pending:
                step()
            if STAGE >= 4:
                for step in epilogue_steps(NCH - 1):
                    step()
            elif STAGE == 3:
                avs.pop(NCH - 1)
                zout = ep_pool.tile([CIN, S], F32, name="zout", tag="zout")
                nc.vector.memset(zout[:], 0.0)
                nc.sync.dma_start(out_d[:], zout[:])

    nc.compile()
    return nc


def _get_compiled():
    global _compiled
    if _compiled is None:
        _compiled = _build()
    return _compiled


def kernel(input, w_qkv, w_out, b_out):
    import ml_dtypes
    input = np.asarray(input, dtype=np.float32)
    w_qkv = np.asarray(w_qkv, dtype=np.float32)
    w_out = np.asarray(w_out, dtype=np.float32)
    b_out = np.asarray(b_out, dtype=np.float32)
    b, x, y, z, c = input.shape
    assert (b, x, y, z, c) == (2, 16, 16, 16, 64)
    hid = HEADS * DH

    xa_by_batch = []
    for bb in range(b):
        xf = input[bb].reshape(S, CIN)
        aug = np.concatenate([xf, np.ones((S, 1), np.float32)], axis=1)
        xa = np.ascontiguousarray(
            aug.reshape(NB, 128, CIN + 1).transpose(1, 0, 2)
        ).astype(ml_dtypes.bfloat16)
        xa_by_batch.append(xa)

    in_maps = []
    xT_by_batch = []
    for bb in range(b):
        xf = input[bb].reshape(S, CIN).T
        xT_by_batch.append(np.ascontiguousarray(
            np.vstack([xf, np.ones((1, S), np.float32)])))

    for core in range(8):
        bb, h = divmod(core, HEADS)
        xT = xT_by_batch[bb]
        wq = np.tile(w_qkv[:, h * DH:(h + 1) * DH], (1, 4))
        wk = np.tile(w_qkv[:, hid + h * DH: hid + (h + 1) * DH], (1, 4))
        wv = np.ascontiguousarray(w_qkv[:, 2 * hid + h * DH: 2 * hid + (h + 1) * DH])
        wo = np.vstack([w_out[h * DH:(h + 1) * DH, :], b_out[None, :] / HEADS])
        wq = np.ascontiguousarray(wq)
        wk = np.ascontiguousarray(wk)
        in_maps.append({
            "xT": xT,
            "xa": xa_by_batch[bb],
            "wq": wq,
            "wk": wk,
            "wqf": wq.copy(),
            "wkf": wk.copy(),
            "wv": wv,
            "wo": np.ascontiguousarray(wo),
        })

    global _last_in_maps
    _last_in_maps = in_maps
    nc = _get_compiled()
    res = run_bass_kernel_spmd(nc, in_maps, core_ids=list(range(8)))
    out = np.zeros((b, S, CIN), dtype=np.float32)
    for core in range(8):
        bb = core // HEADS
        num = res.results[core]["out"]          # [64, S], unnormalized
        l = res.results[core]["ldenom"][0]      # [S]
        out[bb] += (num / l[None, :]).T
    return out.reshape(b, x, y, z, CIN)


if __name__ == "__main__":
    rng = np.random.default_rng(0)
    inp = rng.standard_normal((2, 16, 16, 16, 64), dtype=np.float32)
    wqkv = rng.standard_normal((64, 384), dtype=np.float32) / 8.0
    wout = rng.standard_normal((128, 64), dtype=np.float32) / np.sqrt(128)
    bout = np.zeros(64, dtype=np.float32)
    o = kernel(inp, wqkv, wout, bout)
    print("kernel output shape:", o.shape)


# revision 26
# speedup vs baseline: 1.2378x; 1.2324x over previous
"""Trainium2 Bass kernel for 3D volume attention (b=2, x=y=z=16, c=64,
heads=4, dim_head=32, qk-standardize over sequence, scale=16).

Sharding: batch*heads = 8 (b,h) pairs -> 8 NeuronCores, one pair per core.
Host pre-transposes x and pre-slices per-head weights; host sums the 4
head-partials per batch (pure unshard-reduce) and reshapes.

Per-core pipeline (s=4096, d=32). v2 rewrite of the two-pass softmax
kernel, tuned from a perfetto trace of v1 (307us):
  prologue: standardization stats computed via tiny PE matmuls on the
            Gram matrix G = [x|1]^T [x|1] (sumsq_d = w_d^T G w_d,
            mean from the ones column) instead of serial ACT Square
            passes; rsqrt via exp(-0.5 ln v) + one Newton step so the
            WHOLE kernel uses only the natural_log_exp ACT table set
            (no table thrash); projections drain PSUM directly to the
            standardized bf16 qA (ACT) / kA (DVE) replicas; f32-hat
            rows 0:32 drained separately for the hi/lo residuals
            (gpsimd subtract); input DMA chunked so the PE starts
            early; bf16 v^T/identity transposes.
  pass A  (S[i,j], 2-way row-tiled concurrent matmul pairs): per-quarter
          DVE reduce_max; chunk 0 peeled with a dedicated 4-deep PSUM
          ring, quarters 0,1 exact max on DVE + quarters 2,3 lse bound
          on ACT (16*ln(sum exp(s/16)) - 40 >= max-40).
  pass B  (S^T[j,i]): K=97 bf16 matmul ([khi;khi;klo;-1] x
          [qhi;qlo;qhi;mhat]) -> ACT exp -> bf16 P^T.
  AV:     2-way COLUMN-TILED concurrent pairs: even j-blocks accumulate
          P^T @ [v|1] into PSUM partitions 0:33 (tile_position (0,0)),
          odd j-blocks into partitions 64:97 (tile_position (0,64));
          the epilogue adds the halves. Halves the AV stream time.
  out:    per chunk: DVE add of the two AV halves -> SBUF, project with
          [w_out_h ; b_out/4], DVE copy, DMA out. Host divides by the
          returned softmax denominator during unshard.

Software pipelining: AV lags exp by two iterations (pairs), pass A for
chunk c+1 interleaved one quarter per pass-B iteration. PSUM: 3x[128,1024]
shared ring + [128,1024] col-tiled AV accumulator (8 banks exactly).
"""
import os
import sys
from contextlib import ExitStack

import numpy as np

_PROBLEM_DIR = os.path.dirname(os.path.abspath(__file__))
if _PROBLEM_DIR not in sys.path:
    sys.path.insert(0, _PROBLEM_DIR)

import concourse.bass as bass
import concourse.tile as tile
from concourse import bacc, mybir
from concourse.bass_utils import run_bass_kernel_spmd

F32 = mybir.dt.float32
F32R = mybir.dt.float32r
BF16 = mybir.dt.bfloat16
AF = mybir.ActivationFunctionType
ALU = mybir.AluOpType

HEADS = 4
DH = 32          # dim head
CIN = 64         # input channels
S = 4096         # sequence (16^3)
SCALE = 16.0
EPS = 1e-5
NB = S // 128    # 32 j blocks
NCH = 4          # i chunks
CHUNK = 1024
KP = 97          # 3*32 pair rows + 1 aug row

_compiled = None
STAGE = int(os.environ.get("STAGE", "4"))  # 1=prologue 2=+peel 3=+main-loop 4=full


def _build():
    nc = bacc.Bacc("TRN2", target_bir_lowering=False, debug=False, num_devices=8)
    xT_d = nc.dram_tensor("xT", [CIN + 1, S], F32R, kind="ExternalInput").ap()
    xa_d = nc.dram_tensor("xa", [128, NB, CIN + 1], BF16, kind="ExternalInput").ap()
    wq_d = nc.dram_tensor("wq", [CIN, 128], F32R, kind="ExternalInput").ap()
    wk_d = nc.dram_tensor("wk", [CIN, 128], F32R, kind="ExternalInput").ap()
    wqf_d = nc.dram_tensor("wqf", [CIN, 128], F32, kind="ExternalInput").ap()
    wkf_d = nc.dram_tensor("wkf", [CIN, 128], F32, kind="ExternalInput").ap()
    wv_d = nc.dram_tensor("wv", [CIN, DH], F32R, kind="ExternalInput").ap()
    wo_d = nc.dram_tensor("wo", [DH + 1, CIN], F32R, kind="ExternalInput").ap()
    out_d = nc.dram_tensor("out", [CIN, S], F32, kind="ExternalOutput").ap()
    # softmax denominator per column; host divides during unshard (standard
    # split-softmax partial combination)
    l_d = nc.dram_tensor("ldenom", [1, S], F32R, kind="ExternalOutput").ap()

    with tile.TileContext(nc) as tc, ExitStack() as ctx:
        per = ctx.enter_context(tc.tile_pool(name="per", bufs=1))

        # ---- persistent SBUF ----
        wo_r = per.tile([97, CIN], F32R)  # wo at rows 0:33 AND 64:97
        qA = per.tile([128, S], BF16)          # 4 replicated bands of qhat*16
        kA = per.tile([128, S], BF16)          # 4 replicated bands of khat
        kP = per.tile([KP, S], BF16)           # [khi; khi; klo; -1]
        vaug = per.tile([128, NB, DH + 1], BF16)   # per j-block [v | 1]
        qPc = [per.tile([KP, CHUNK], BF16, name=f"qPc{c}") for c in range(NCH)]
        mcolT = [per.tile([128, 8], F32, name=f"mcolT{c}") for c in range(NCH)]
        neg1 = per.tile([128, DH], BF16)
        identb = per.tile([128, 128], BF16)
        identf = per.tile([128, 128], F32)

        with tc.tile_pool(name="prow", bufs=1) as prow:
            # ---- input DMAs ----
            xa_sb = prow.tile([128, NB, CIN + 1], BF16)
            nc.sync.dma_start(xa_sb[:], xa_d[:])
            # augmented projection weights: row 64 = -mu (written on device)
            wq_aug = prow.tile([CIN + 1, 128], F32R)
            wk_aug = prow.tile([CIN + 1, 128], F32R)
            wqf = prow.tile([CIN, 128], F32)
            wkf = prow.tile([CIN, 128], F32)
            wv_r = prow.tile([CIN, DH], F32R)
            nc.sync.dma_start(wq_aug[0:CIN, :], wq_d[:])
            nc.sync.dma_start(wk_aug[0:CIN, :], wk_d[:])
            nc.sync.dma_start(wqf[:], wqf_d[:])
            nc.sync.dma_start(wkf[:], wkf_d[:])
            nc.sync.dma_start(wv_r[:], wv_d[:])
            nc.sync.dma_start(wo_r[0:DH + 1, :], wo_d[:])
            nc.sync.dma_start(wo_r[64:97, :], wo_d[:])
            xTr = prow.tile([CIN + 1, S], F32R)  # row 64 = ones (host-side)
            nc.sync.dma_start(xTr[:], xT_d[:])

            from concourse.masks import make_identity
            make_identity(nc, identb[:])
            make_identity(nc, identf[:])

            # ---- stats via Gram matrix: G = [x|1]^T [x|1]  ([64, 65]) ----
            ones64 = prow.tile([CIN, 1], F32)
            nc.vector.memset(ones64[:], 1.0)
            mu2 = prow.tile([128, 2], F32)    # col 0 = q, col 1 = k
            ex22 = prow.tile([128, 2], F32)
            with tc.tile_pool(name="gp", bufs=1, space="PSUM") as gp:
                psG = gp.tile([CIN, CIN + 1], F32, name="psG")
                for b in range(NB):
                    nc.tensor.matmul(psG[:], xa_sb[:, b, 0:CIN], xa_sb[:, b, :],
                                     start=(b == 0), stop=(b == NB - 1))
                Gsb = prow.tile([CIN, CIN + 1], F32R)
                nc.vector.tensor_copy(Gsb[:], psG[:])

                def stats_for(w_r, w_f, col, nm):
                    # mu = w^T sx / S ; ex2 = diag(w^T G w) / S
                    # (f32r matmuls need a moving free dim >= 2: use a 2-col
                    # window of Gsb whose second column is sx)
                    psMu = gp.tile([128, 2], F32, name=f"psMu{nm}")
                    nc.tensor.matmul(psMu[:], w_r[:], Gsb[:, CIN - 1:CIN + 1],
                                     start=True, stop=True)
                    psH = gp.tile([CIN, 128], F32, name=f"psH{nm}")
                    nc.tensor.matmul(psH[:], Gsb[:, 0:CIN], w_r[:],
                                     start=True, stop=True)
                    Hs = prow.tile([CIN, 128], F32, name=f"Hs{nm}")
                    nc.vector.tensor_copy(Hs[:], psH[:])
                    prod = prow.tile([CIN, 128], F32, name=f"prod{nm}")
                    nc.vector.tensor_tensor(out=prod[:], in0=Hs[:], in1=w_f[:],
                                            op=ALU.mult)
                    psSq = gp.tile([128, 1], F32, name=f"psSq{nm}")
                    nc.tensor.matmul(psSq[:], prod[:], ones64[:],
                                     start=True, stop=True)
                    nc.vector.tensor_scalar_mul(mu2[:, col:col + 1],
                                                psMu[:, 1:2], 1.0 / S)
                    nc.vector.tensor_scalar_mul(ex22[:, col:col + 1],
                                                psSq[:], 1.0 / S)

                stats_for(wq_aug[0:CIN, :], wqf, 0, "q")
                stats_for(wk_aug[0:CIN, :], wkf, 1, "k")

                # -mu rows for the mean-subtracting projections (two separate
                # transposes so both rows land at partition 0 -- DVE reads
                # need a 32-aligned partition base)
                psmuT = gp.tile([1, 256], F32, name="psmuT")
                nc.tensor.transpose(psmuT[0:1, 0:128], mu2[:, 0:1], identf[:])
                nc.tensor.transpose(psmuT[0:1, 128:256], mu2[:, 1:2],
                                    identf[:])
                nc.vector.tensor_scalar_mul(wq_aug[CIN:CIN + 1, :],
                                            psmuT[0:1, 0:128], -1.0)
                nc.vector.tensor_scalar_mul(wk_aug[CIN:CIN + 1, :],
                                            psmuT[0:1, 128:256], -1.0)

            # rstd for q (x16 fold) and k, batched so ln/exp each cost one
            # ACT table-set switch; one Newton polish step
            musq2 = prow.tile([128, 2], F32)
            nc.vector.tensor_tensor(out=musq2[:], in0=mu2[:], in1=mu2[:],
                                    op=ALU.mult)
            vareps2 = prow.tile([128, 2], F32)
            nc.vector.tensor_tensor(out=vareps2[:], in0=ex22[:], in1=musq2[:],
                                    op=ALU.subtract)
            nc.vector.tensor_scalar_add(vareps2[:], vareps2[:], EPS)
            lnv2 = prow.tile([128, 2], F32)
            nc.scalar.activation(lnv2[:], vareps2[:], AF.Ln)
            r02 = prow.tile([128, 2], F32)
            nc.scalar.activation(r02[:], lnv2[:], AF.Exp, scale=-0.5)
            r0sq2 = prow.tile([128, 2], F32)
            nc.vector.tensor_tensor(out=r0sq2[:], in0=r02[:], in1=r02[:],
                                    op=ALU.mult)
            h2 = prow.tile([128, 2], F32)
            nc.vector.tensor_tensor(out=h2[:], in0=r0sq2[:], in1=vareps2[:],
                                    op=ALU.mult)
            w2 = prow.tile([128, 2], F32)
            nc.vector.tensor_scalar(out=w2[:], in0=h2[:], scalar1=-0.5,
                                    scalar2=1.5, op0=ALU.mult, op1=ALU.add)
            fold2 = prow.tile([128, 2], F32)
            nc.vector.memset(fold2[:, 0:1], SCALE)
            nc.vector.memset(fold2[:, 1:2], 1.0)
            rstd2r = prow.tile([128, 2], F32)
            nc.vector.tensor_tensor(out=rstd2r[:], in0=r02[:], in1=w2[:],
                                    op=ALU.mult)
            rstd2 = prow.tile([128, 2], F32)
            nc.vector.tensor_tensor(out=rstd2[:], in0=rstd2r[:], in1=fold2[:],
                                    op=ALU.mult)

            # ---- projections (mean already subtracted via the -mu row);
            # drains go straight to standardized bf16 + bf16 lo-residuals ----
            qlo_t = prow.tile([DH, S], BF16)
            klo_t = prow.tile([DH, S], BF16)
            vbf = prow.tile([DH, S], BF16)

            with tc.tile_pool(name="props", bufs=2, space="PSUM") as props:
                for half in range(2):
                    ppq = props.tile([128, 4, 512], F32, name=f"ppq{half}",
                                     tag="pp")
                    for n in range(4):
                        sl = bass.ds(2048 * half + 512 * n, 512)
                        nc.tensor.matmul(ppq[:, n, :], wq_aug[:], xTr[:, sl],
                                         start=True, stop=True)
                    nc.scalar.activation(qA[:, bass.ts(half, 2048)], ppq[:],
                                         AF.Identity, scale=rstd2[:, 0:1])
                    # lo residual: (psum*rstd) - qA, fused on DVE
                    nc.vector.scalar_tensor_tensor(
                        out=qlo_t[:, bass.ts(half, 2048)],
                        in0=ppq[0:DH, :, :], scalar=rstd2[0:DH, 0:1],
                        in1=qA[0:DH, bass.ts(half, 2048)],
                        op0=ALU.mult, op1=ALU.subtract)
                    ppk = props.tile([128, 4, 512], F32, name=f"ppk{half}",
                                     tag="pp")
                    for n in range(4):
                        sl = bass.ds(2048 * half + 512 * n, 512)
                        nc.tensor.matmul(ppk[:, n, :], wk_aug[:], xTr[:, sl],
                                         start=True, stop=True)
                    nc.scalar.activation(kA[:, bass.ts(half, 2048)], ppk[:],
                                         AF.Identity, scale=rstd2[:, 1:2])
                    nc.vector.scalar_tensor_tensor(
                        out=klo_t[:, bass.ts(half, 2048)],
                        in0=ppk[0:DH, :, :], scalar=rstd2[0:DH, 1:2],
                        in1=kA[0:DH, bass.ts(half, 2048)],
                        op0=ALU.mult, op1=ALU.subtract)
                # ---- prologue-peel: pass A for chunk 0, emitted BEFORE the v
                # projection so it starts as soon as qA/kA exist. 4-way
                # row-tiled groups into one [128, 2048] tile (4 banks): the
                # j-halves 0,1 get an exact DVE max, halves 2,3 an ACT lse
                # bound (16*ln(sum exp(s/16)) - 40 >= max-40). ----
                if STAGE >= 2:
                    l8all = prow.tile([128, 8], F32, name="l8all")
                    m01all = prow.tile([128, 8], F32, name="m01all")
                    bias25 = prow.tile([128, 1], F32, name="bias25")
                    nc.vector.memset(bias25[:], -25.0)
                    with tc.tile_pool(name="jkp", bufs=2) as jk_pool:
                        for blk in range(8):
                            for qp in range(2):
                                pa = props.tile([128, 2048], F32,
                                                name=f"pa{blk}_{qp}", tag="pp")
                                for r in range(4):
                                    nc.tensor.matmul(
                                        pa[:, bass.ts(r, 512)],
                                        qA[bass.ts(r, 32), bass.ts(blk, 128)],
                                        kA[bass.ts(r, 32),
                                           bass.ds(2048 * qp + 512 * r, 512)],
                                        start=True, stop=True,
                                        tile_position=(32 * r, 0),
                                    )
                                if qp == 0:
                                    nc.vector.reduce_max(
                                        m01all[:, blk:blk + 1], pa[:],
                                        axis=mybir.AxisListType.X)
                                else:
                                    ju = jk_pool.tile([128, 2048], BF16,
                                                      name=f"ju{blk}", tag="ju")
                                    nc.scalar.activation(
                                        ju[:], pa[:], AF.Exp, scale=0.0625,
                                        bias=bias25[:],
                                        accum_out=l8all[:, blk:blk + 1])
                    lnt = prow.tile([128, 8], F32, name="lnt")
                    nc.scalar.activation(lnt[:], l8all[:], AF.Ln)
                    mlse = prow.tile([128, 8], F32, name="mlse")
                    # m = 16*(ln l8' + 25) - 40 = 16*ln l8' + 360
                    nc.vector.tensor_scalar(out=mlse[:], in0=lnt[:],
                                            scalar1=16.0, scalar2=360.0,
                                            op0=ALU.mult, op1=ALU.add)
                    nc.vector.tensor_tensor(out=mcolT[0][:], in0=m01all[:],
                                            in1=mlse[:], op=ALU.max)

                # v projection -> bf16 v (rows 0:32)
                for half in range(2):
                    pv = props.tile([128, 4, 512], F32, name=f"pv{half}",
                                    tag="pp")
                    for n in range(4):
                        nc.tensor.matmul(pv[0:DH, n, :], wv_r[:],
                                         xTr[0:CIN,
                                             bass.ds(2048 * half + 512 * n,
                                                     512)],
                                         start=True, stop=True)
                    if half == 0:
                        nc.scalar.copy(vbf[:, bass.ts(half, 2048)],
                                       pv[0:DH, :, :])
                    else:
                        nc.vector.tensor_copy(vbf[:, bass.ts(half, 2048)],
                                              pv[0:DH, :, :])

                # ---- vaug: PE transposes of bf16 v -> [j, d|1] blocks ----
                nc.gpsimd.memset(vaug[:], 1.0)
                for g in range(8):
                    pt4 = props.tile([128, 4, 512], BF16, name=f"pvt{g}",
                                     tag="pp")
                    for t in range(4):
                        jb = 4 * g + t
                        nc.tensor.transpose(pt4[:, t, 0:DH],
                                            vbf[:, bass.ts(jb, 128)],
                                            identb[0:DH, 0:DH])
                    nc.vector.tensor_copy(vaug[:, 4 * g:4 * g + 4, 0:DH],
                                          pt4[:, :, 0:DH])

            # ---- hi/lo pair tiles ----
            # kP = [khi; khi; klo; -1]; qPc[c] = [qhi; qlo; qhi; mhat]
            nc.sync.dma_start(kP[0:DH, :], kA[0:DH, :])
            nc.sync.dma_start(kP[DH:2 * DH, :], kA[DH:2 * DH, :])
            nc.sync.dma_start(kP[2 * DH:3 * DH, :], klo_t[:])
            for c in range(NCH):
                cs = bass.ts(c, CHUNK)
                nc.sync.dma_start(qPc[c][0:DH, :], qA[0:DH, cs])
                nc.sync.dma_start(qPc[c][2 * DH:3 * DH, :], qA[2 * DH:3 * DH, cs])
                nc.sync.dma_start(qPc[c][DH:2 * DH, :], qlo_t[:, cs])
            # kP row 96 = -1 via tiny memset + reshape DMA
            nc.gpsimd.memset(neg1[:], -1.0)
            nc.sync.dma_start(kP[96:97, :], neg1[:])

        # ================= main loop =================
        with tc.tile_pool(name="uni", bufs=3, space="PSUM") as uni_pool, \
             tc.tile_pool(name="psAV", bufs=1, space="PSUM") as psAV_pool, \
             tc.tile_pool(name="mpp2", bufs=3) as mp_pool, \
             tc.tile_pool(name="ptp", bufs=4) as pt_pool, \
             tc.tile_pool(name="epp", bufs=2) as ep_pool:

            if STAGE <= 2:
                zout = ep_pool.tile([CIN, S], F32, name="zout", tag="zout")
                nc.vector.memset(zout[:], 0.0)
                nc.sync.dma_start(out_d[:], zout[:])

            mparts_t = {}

            def emit_passA_half(blk, qp):
                # 4-way row-tiled concurrent group covering HALF a block's
                # j range (2048 cols) via two uni slots (4 distinct banks)
                if qp == 0:
                    mparts_t[blk] = mp_pool.tile([128, 4], F32, name=f"mp{blk}",
                                                 tag="mparts")
                mp = mparts_t[blk]
                pa0 = uni_pool.tile([128, 1024], F32, name=f"pa{blk}_{qp}a",
                                    tag="uni")
                pa1 = uni_pool.tile([128, 1024], F32, name=f"pa{blk}_{qp}b",
                                    tag="uni")
                for r in range(4):
                    dst = pa0 if r < 2 else pa1
                    nc.tensor.matmul(
                        dst[:, bass.ts(r % 2, 512)],
                        qA[bass.ts(r, 32), bass.ts(blk, 128)],
                        kA[bass.ts(r, 32), bass.ds(2048 * qp + 512 * r, 512)],
                        start=True, stop=True,
                        tile_position=(32 * r, 0),
                    )
                nc.vector.reduce_max(mp[:, 2 * qp:2 * qp + 1], pa0[:],
                                     axis=mybir.AxisListType.X)
                nc.vector.reduce_max(mp[:, 2 * qp + 1:2 * qp + 2], pa1[:],
                                     axis=mybir.AxisListType.X)
                if qp == 1:
                    mparts_t.pop(blk)
                    nc.vector.reduce_max(
                        mcolT[blk // 8][:, (blk % 8):(blk % 8) + 1], mp[:],
                        axis=mybir.AxisListType.X)

            def emit_mhat(c):
                # 8 max columns -> PE transpose -> bf16 row -> reshape DMA into
                # row 96 of qPc[c]
                psm = uni_pool.tile([128, 1024], F32, name=f"psm{c}", tag="uni")
                nc.tensor.transpose(psm[0:8, 0:128], mcolT[c][:], identf[:])
                m8 = ep_pool.tile([8, 128], BF16, name=f"m8_{c}", tag="m8")
                nc.vector.tensor_copy(m8[:], psm[0:8, 0:128])
                # explicit 3D dst AP pins descriptor order (block-major)
                nc.sync.dma_start(
                    qPc[c][96:97, :].rearrange("a (b c) -> a b c", b=8), m8[:])

            avs = {}
            pts = {}

            def emit_AV_pair(c, jb0):
                # col-tiled concurrent pairs: even jb -> partitions 0:33 at
                # tile_position (0,0); odd jb -> partitions 64:97 at (0,64).
                # Adjacent matmuls must hit DIFFERENT PSUM banks to stream
                # concurrently, so interleave as (E-h0 | O-h1), (O-h0 | E-h1).
                avh = avs[c]
                pte = pts.pop(jb0)
                pto = pts.pop(jb0 + 1)
                st = (jb0 == 0)
                sp = (jb0 == NB - 2)
                nc.tensor.matmul(avh[0:DH + 1, 0:512], vaug[:, jb0, :],
                                 pte[:, 0:512], start=st, stop=sp,
                                 tile_position=(0, 0))
                nc.tensor.matmul(avh[64:64 + DH + 1, 512:1024],
                                 vaug[:, jb0 + 1, :], pto[:, 512:1024],
                                 start=st, stop=sp, tile_position=(0, 64))
                nc.tensor.matmul(avh[64:64 + DH + 1, 0:512],
                                 vaug[:, jb0 + 1, :], pto[:, 0:512],
                                 start=st, stop=sp, tile_position=(0, 64))
                nc.tensor.matmul(avh[0:DH + 1, 512:1024], vaug[:, jb0, :],
                                 pte[:, 512:1024], start=st, stop=sp,
                                 tile_position=(0, 0))

            def epilogue_steps(c):
                # chunk epilogue split into closures, one per early iteration
                # of the next chunk, to spread PSUM-ring + DVE pressure.
                # Emits the UNNORMALIZED projection wo^T @ [av; l] plus the
                # denominator row; the host divides during unshard.
                avh = avs.pop(c)
                avsb = ep_pool.tile([DH + 1, CHUNK], F32R, name=f"avsb{c}",
                                    tag="avsb", bufs=4)

                def s0():
                    # combine the two col-tiled AV halves (DVE can read only
                    # one PSUM operand per instruction)
                    nc.vector.tensor_copy(avsb[:], avh[0:DH + 1, :])
                    nc.vector.tensor_tensor(out=avsb[:], in0=avsb[:],
                                            in1=avh[64:64 + DH + 1, :],
                                            op=ALU.add)
                    nc.sync.dma_start(l_d[:, bass.ts(c, CHUNK)], avsb[32:33, :])

                def seg_step(seg):
                    def s():
                        sg = bass.ts(seg, 512)
                        psY = uni_pool.tile([128, 1024], F32, name=f"psY{c}_{seg}",
                                            tag="uni")
                        nc.tensor.matmul(psY[0:CIN, 0:512], wo_r[0:DH + 1, :],
                                         avsb[:, sg], start=True, stop=True)
                        ysb = ep_pool.tile([CIN, 512], F32, name=f"ysb{c}_{seg}",
                                           tag="ysb")
                        nc.vector.tensor_copy(ysb[:], psY[0:CIN, 0:512])
                        nc.sync.dma_start(out_d[:, bass.ds(CHUNK * c + 512 * seg,
                                                           512)], ysb[:])
                    return s

                return [s0, seg_step(0), seg_step(1)]

            if STAGE == 2:
                zout = ep_pool.tile([CIN, S], F32, name="zout", tag="zout")
                nc.vector.memset(zout[:], 0.0)
                nc.sync.dma_start(out_d[:], zout[:])
            pending = []
            for c in range(NCH if STAGE >= 3 else 0):
                if c == 0:
                    emit_mhat(0)
                if c > 0 and STAGE >= 4:
                    steps = epilogue_steps(c - 1)
                    # step 0 (avh halves -> SBUF add) must precede the
                    # reallocation of the single-buffer AV accumulator below
                    steps[0]()
                    pending = steps[1:]
                elif c > 0:
                    avs.pop(c - 1)
                avs[c] = psAV_pool.tile([128, CHUNK], F32, name=f"av{c}",
                                        tag="av")
                for jb in range(NB):
                    if c + 1 < NCH:
                        if jb % 2 == 0:
                            emit_passA_half(8 * (c + 1) + jb // 4, (jb % 4) // 2)
                        if jb == NB - 1:
                            # next chunk's mhat row, well before its pass B
                            emit_mhat(c + 1)
                    psB = uni_pool.tile([128, CHUNK], F32, name=f"psB{c}_{jb}",
                                        tag="uni")
                    for hf in range(2):
                        nc.tensor.matmul(psB[:, bass.ts(hf, 512)],
                                         kP[:, bass.ts(jb, 128)],
                                         qPc[c][:, bass.ts(hf, 512)],
                                         start=True, stop=True)
                    pt = pt_pool.tile([128, CHUNK], BF16, name=f"pt{c}_{jb}",
                                      tag="pt")
                    nc.scalar.activation(pt[:], psB[:], AF.Exp)
                    pts[jb] = pt
                    if jb >= 3 and jb % 2 == 1:
                        emit_AV_pair(c, jb - 3)
                    if pending and jb >= 2:
                        pending.pop(0)()
                emit_AV_pair(c, NB - 2)
            for step in pending:
                step()
            if STAGE >= 4:
                for step in epilogue_steps(NCH - 1):
                    step()
            elif STAGE == 3:
                avs.pop(NCH - 1)
                zout = ep_pool.tile([CIN, S], F32, name="zout", tag="zout")
                nc.vector.memset(zout[:], 0.0)
                nc.sync.dma_start(out_d[:], zout[:])

    nc.compile()
    return nc


def _get_compiled():
    global _compiled
    if _compiled is None:
        _compiled = _build()
    return _compiled


def kernel(input, w_qkv, w_out, b_out):
    import ml_dtypes
    input = np.asarray(input, dtype=np.float32)
    w_qkv = np.asarray(w_qkv, dtype=np.float32)
    w_out = np.asarray(w_out, dtype=np.float32)
    b_out = np.asarray(b_out, dtype=np.float32)
    b, x, y, z, c = input.shape
    assert (b, x, y, z, c) == (2, 16, 16, 16, 64)
    hid = HEADS * DH

    xa_by_batch = []
    for bb in range(b):
        xf = input[bb].reshape(S, CIN)
        aug = np.concatenate([xf, np.ones((S, 1), np.float32)], axis=1)
        xa = np.ascontiguousarray(
            aug.reshape(NB, 128, CIN + 1).transpose(1, 0, 2)
        ).astype(ml_dtypes.bfloat16)
        xa_by_batch.append(xa)

    in_maps = []
    xT_by_batch = []
    for bb in range(b):
        xf = input[bb].reshape(S, CIN).T
        xT_by_batch.append(np.ascontiguousarray(
            np.vstack([xf, np.ones((1, S), np.float32)])))

    for core in range(8):
        bb, h = divmod(core, HEADS)
        xT = xT_by_batch[bb]
        wq = np.tile(w_qkv[:, h * DH:(h + 1) * DH], (1, 4))
        wk = np.tile(w_qkv[:, hid + h * DH: hid + (h + 1) * DH], (1, 4))
        wv = np.ascontiguousarray(w_qkv[:, 2 * hid + h * DH: 2 * hid + (h + 1) * DH])
        wo = np.vstack([w_out[h * DH:(h + 1) * DH, :], b_out[None, :] / HEADS])
        wq = np.ascontiguousarray(wq)
        wk = np.ascontiguousarray(wk)
        in_maps.append({
            "xT": xT,
            "xa": xa_by_batch[bb],
            "wq": wq,
            "wk": wk,
            "wqf": wq.copy(),
            "wkf": wk.copy(),
            "wv": wv,
            "wo": np.ascontiguousarray(wo),
        })

    global _last_in_maps
    _last_in_maps = in_maps
    nc = _get_compiled()
    res = run_bass_kernel_spmd(nc, in_maps, core_ids=list(range(8)))
    out = np.zeros((b, S, CIN), dtype=np.float32)
    for core in range(8):
        bb = core // HEADS
        num = res.results[core]["out"]          # [64, S], unnormalized
        l = res.results[core]["ldenom"][0]      # [S]
        out[bb] += (num / l[None, :]).T
    return out.reshape(b, x, y, z, CIN)


if __name__ == "__main__":
    rng = np.random.default_rng(0)
    inp = rng.standard_normal((2, 16, 16, 16, 64), dtype=np.float32)
    wqkv = rng.standard_normal((64, 384), dtype=np.float32) / 8.0
    wout = rng.standard_normal((128, 64), dtype=np.float32) / np.sqrt(128)
    bout = np.zeros(64, dtype=np.float32)
    o = kernel(inp, wqkv, wout, bout)
    print("kernel output shape:", o.shape)


# revision 27
# speedup vs baseline: 1.2649x; 1.0219x over previous
"""Trainium2 Bass kernel for 3D volume attention (b=2, x=y=z=16, c=64,
heads=4, dim_head=32, qk-standardize over sequence, scale=16).

Sharding: batch*heads = 8 (b,h) pairs -> 8 NeuronCores, one pair per core.
Host pre-transposes x and pre-slices per-head weights; host sums the 4
head-partials per batch (pure unshard-reduce) and reshapes.

Per-core pipeline (s=4096, d=32). v2 rewrite of the two-pass softmax
kernel, tuned from a perfetto trace of v1 (307us):
  prologue: standardization stats computed via tiny PE matmuls on the
            Gram matrix G = [x|1]^T [x|1] (sumsq_d = w_d^T G w_d,
            mean from the ones column) instead of serial ACT Square
            passes; rsqrt via exp(-0.5 ln v) + one Newton step so the
            WHOLE kernel uses only the natural_log_exp ACT table set
            (no table thrash); projections drain PSUM directly to the
            standardized bf16 qA (ACT) / kA (DVE) replicas; f32-hat
            rows 0:32 drained separately for the hi/lo residuals
            (gpsimd subtract); input DMA chunked so the PE starts
            early; bf16 v^T/identity transposes.
  pass A  (S[i,j], 2-way row-tiled concurrent matmul pairs): per-quarter
          DVE reduce_max; chunk 0 peeled with a dedicated 4-deep PSUM
          ring, quarters 0,1 exact max on DVE + quarters 2,3 lse bound
          on ACT (16*ln(sum exp(s/16)) - 40 >= max-40).
  pass B  (S^T[j,i]): K=97 bf16 matmul ([khi;khi;klo;-1] x
          [qhi;qlo;qhi;mhat]) -> ACT exp -> bf16 P^T.
  AV:     2-way COLUMN-TILED concurrent pairs: even j-blocks accumulate
          P^T @ [v|1] into PSUM partitions 0:33 (tile_position (0,0)),
          odd j-blocks into partitions 64:97 (tile_position (0,64));
          the epilogue adds the halves. Halves the AV stream time.
  out:    per chunk: DVE add of the two AV halves -> SBUF, project with
          [w_out_h ; b_out/4], DVE copy, DMA out. Host divides by the
          returned softmax denominator during unshard.

Software pipelining: AV lags exp by two iterations (pairs), pass A for
chunk c+1 interleaved one quarter per pass-B iteration. PSUM: 3x[128,1024]
shared ring + [128,1024] col-tiled AV accumulator (8 banks exactly).
"""
import os
import sys
from contextlib import ExitStack

import numpy as np

_PROBLEM_DIR = os.path.dirname(os.path.abspath(__file__))
if _PROBLEM_DIR not in sys.path:
    sys.path.insert(0, _PROBLEM_DIR)

import concourse.bass as bass
import concourse.tile as tile
from concourse import bacc, mybir
from concourse.bass_utils import run_bass_kernel_spmd

F32 = mybir.dt.float32
F32R = mybir.dt.float32r
BF16 = mybir.dt.bfloat16
AF = mybir.ActivationFunctionType
ALU = mybir.AluOpType

HEADS = 4
DH = 32          # dim head
CIN = 64         # input channels
S = 4096         # sequence (16^3)
SCALE = 16.0
EPS = 1e-5
NB = S // 128    # 32 j blocks
NCH = 4          # i chunks
CHUNK = 1024
KP = 97          # 3*32 pair rows + 1 aug row

_compiled = None
STAGE = int(os.environ.get("STAGE", "4"))  # 1=prologue 2=+peel 3=+main-loop 4=full


def _build():
    nc = bacc.Bacc("TRN2", target_bir_lowering=False, debug=False, num_devices=8)
    xT_d = nc.dram_tensor("xT", [CIN + 1, S], F32R, kind="ExternalInput").ap()
    xa_d = nc.dram_tensor("xa", [128, NB, CIN + 1], BF16, kind="ExternalInput").ap()
    wq_d = nc.dram_tensor("wq", [CIN, 128], F32R, kind="ExternalInput").ap()
    wk_d = nc.dram_tensor("wk", [CIN, 128], F32R, kind="ExternalInput").ap()
    wqf_d = nc.dram_tensor("wqf", [CIN, 128], F32, kind="ExternalInput").ap()
    wkf_d = nc.dram_tensor("wkf", [CIN, 128], F32, kind="ExternalInput").ap()
    wv_d = nc.dram_tensor("wv", [CIN, DH], F32R, kind="ExternalInput").ap()
    wo_d = nc.dram_tensor("wo", [DH + 1, CIN], F32R, kind="ExternalInput").ap()
    out_d = nc.dram_tensor("out", [CIN, S], F32, kind="ExternalOutput").ap()
    # softmax denominator per column; host divides during unshard (standard
    # split-softmax partial combination)
    l_d = nc.dram_tensor("ldenom", [1, S], F32R, kind="ExternalOutput").ap()

    with tile.TileContext(nc) as tc, ExitStack() as ctx:
        per = ctx.enter_context(tc.tile_pool(name="per", bufs=1))

        # ---- persistent SBUF ----
        wo_r = per.tile([97, CIN], F32R)  # wo at rows 0:33 AND 64:97
        qA = per.tile([128, S], BF16)          # 4 replicated bands of qhat*16
        kA = per.tile([128, S], BF16)          # 4 replicated bands of khat
        kP = per.tile([KP, S], BF16)           # [khi; khi; klo; -1]
        vaug = per.tile([128, NB, DH + 1], BF16)   # per j-block [v | 1]
        qPc = [per.tile([KP, CHUNK], BF16, name=f"qPc{c}") for c in range(NCH)]
        mcolT = [per.tile([128, 8], F32, name=f"mcolT{c}") for c in range(NCH)]
        neg1 = per.tile([128, DH], BF16)
        identb = per.tile([128, 128], BF16)
        identf = per.tile([128, 128], F32)

        with tc.tile_pool(name="prow", bufs=1) as prow:
            # ---- input DMAs ----
            xa_sb = prow.tile([128, NB, CIN + 1], BF16)
            nc.sync.dma_start(xa_sb[:], xa_d[:])
            # augmented projection weights: row 64 = -mu (written on device)
            wq_aug = prow.tile([CIN + 1, 128], F32R)
            wk_aug = prow.tile([CIN + 1, 128], F32R)
            wqf = prow.tile([CIN, 128], F32)
            wkf = prow.tile([CIN, 128], F32)
            wv_r = prow.tile([CIN, DH], F32R)
            nc.sync.dma_start(wq_aug[0:CIN, :], wq_d[:])
            nc.sync.dma_start(wk_aug[0:CIN, :], wk_d[:])
            nc.sync.dma_start(wqf[:], wqf_d[:])
            nc.sync.dma_start(wkf[:], wkf_d[:])
            nc.sync.dma_start(wv_r[:], wv_d[:])
            nc.sync.dma_start(wo_r[0:DH + 1, :], wo_d[:])
            nc.sync.dma_start(wo_r[64:97, :], wo_d[:])
            xTr = prow.tile([CIN + 1, S], F32R)  # row 64 = ones (host-side)
            nc.sync.dma_start(xTr[:], xT_d[:])

            from concourse.masks import make_identity
            make_identity(nc, identb[:])
            make_identity(nc, identf[:])

            # ---- stats via Gram matrix: G = [x|1]^T [x|1]  ([64, 65]) ----
            ones64 = prow.tile([CIN, 1], F32)
            nc.vector.memset(ones64[:], 1.0)
            mu2 = prow.tile([128, 2], F32)    # col 0 = q, col 1 = k
            ex22 = prow.tile([128, 2], F32)
            with tc.tile_pool(name="gp", bufs=1, space="PSUM") as gp:
                psG = gp.tile([CIN, CIN + 1], F32, name="psG")
                for b in range(NB):
                    nc.tensor.matmul(psG[:], xa_sb[:, b, 0:CIN], xa_sb[:, b, :],
                                     start=(b == 0), stop=(b == NB - 1))
                Gsb = prow.tile([CIN, CIN + 1], F32R)
                nc.vector.tensor_copy(Gsb[:], psG[:])

                def stats_for(w_r, w_f, col, nm):
                    # mu = w^T sx / S ; ex2 = diag(w^T G w) / S
                    # (f32r matmuls need a moving free dim >= 2: use a 2-col
                    # window of Gsb whose second column is sx)
                    psMu = gp.tile([128, 2], F32, name=f"psMu{nm}")
                    nc.tensor.matmul(psMu[:], w_r[:], Gsb[:, CIN - 1:CIN + 1],
                                     start=True, stop=True)
                    psH = gp.tile([CIN, 128], F32, name=f"psH{nm}")
                    nc.tensor.matmul(psH[:], Gsb[:, 0:CIN], w_r[:],
                                     start=True, stop=True)
                    Hs = prow.tile([CIN, 128], F32, name=f"Hs{nm}")
                    nc.vector.tensor_copy(Hs[:], psH[:])
                    prod = prow.tile([CIN, 128], F32, name=f"prod{nm}")
                    nc.vector.tensor_tensor(out=prod[:], in0=Hs[:], in1=w_f[:],
                                            op=ALU.mult)
                    psSq = gp.tile([128, 1], F32, name=f"psSq{nm}")
                    nc.tensor.matmul(psSq[:], prod[:], ones64[:],
                                     start=True, stop=True)
                    nc.vector.tensor_scalar_mul(mu2[:, col:col + 1],
                                                psMu[:, 1:2], 1.0 / S)
                    nc.vector.tensor_scalar_mul(ex22[:, col:col + 1],
                                                psSq[:], 1.0 / S)

                stats_for(wq_aug[0:CIN, :], wqf, 0, "q")
                stats_for(wk_aug[0:CIN, :], wkf, 1, "k")

                # -mu rows for the mean-subtracting projections (two separate
                # transposes so both rows land at partition 0 -- DVE reads
                # need a 32-aligned partition base)
                psmuT = gp.tile([1, 256], F32, name="psmuT")
                nc.tensor.transpose(psmuT[0:1, 0:128], mu2[:, 0:1], identf[:])
                nc.tensor.transpose(psmuT[0:1, 128:256], mu2[:, 1:2],
                                    identf[:])
                nc.vector.tensor_scalar_mul(wq_aug[CIN:CIN + 1, :],
                                            psmuT[0:1, 0:128], -1.0)
                nc.vector.tensor_scalar_mul(wk_aug[CIN:CIN + 1, :],
                                            psmuT[0:1, 128:256], -1.0)

            # rstd for q (x16 fold) and k, batched so ln/exp each cost one
            # ACT table-set switch; one Newton polish step
            musq2 = prow.tile([128, 2], F32)
            nc.vector.tensor_tensor(out=musq2[:], in0=mu2[:], in1=mu2[:],
                                    op=ALU.mult)
            vareps2 = prow.tile([128, 2], F32)
            nc.vector.tensor_tensor(out=vareps2[:], in0=ex22[:], in1=musq2[:],
                                    op=ALU.subtract)
            nc.vector.tensor_scalar_add(vareps2[:], vareps2[:], EPS)
            lnv2 = prow.tile([128, 2], F32)
            nc.scalar.activation(lnv2[:], vareps2[:], AF.Ln)
            r02 = prow.tile([128, 2], F32)
            nc.scalar.activation(r02[:], lnv2[:], AF.Exp, scale=-0.5)
            r0sq2 = prow.tile([128, 2], F32)
            nc.vector.tensor_tensor(out=r0sq2[:], in0=r02[:], in1=r02[:],
                                    op=ALU.mult)
            h2 = prow.tile([128, 2], F32)
            nc.vector.tensor_tensor(out=h2[:], in0=r0sq2[:], in1=vareps2[:],
                                    op=ALU.mult)
            w2 = prow.tile([128, 2], F32)
            nc.vector.tensor_scalar(out=w2[:], in0=h2[:], scalar1=-0.5,
                                    scalar2=1.5, op0=ALU.mult, op1=ALU.add)
            fold2 = prow.tile([128, 2], F32)
            nc.vector.memset(fold2[:, 0:1], SCALE)
            nc.vector.memset(fold2[:, 1:2], 1.0)
            rstd2r = prow.tile([128, 2], F32)
            nc.vector.tensor_tensor(out=rstd2r[:], in0=r02[:], in1=w2[:],
                                    op=ALU.mult)
            rstd2 = prow.tile([128, 2], F32)
            nc.vector.tensor_tensor(out=rstd2[:], in0=rstd2r[:], in1=fold2[:],
                                    op=ALU.mult)

            # ---- projections (mean already subtracted via the -mu row);
            # drains go straight to standardized bf16 + bf16 lo-residuals ----
            qlo_t = prow.tile([DH, S], BF16)
            klo_t = prow.tile([DH, S], BF16)
            vbf = prow.tile([DH, S], BF16)

            with tc.tile_pool(name="props", bufs=2, space="PSUM") as props:
                def proj_half(half):
                    ppq = props.tile([128, 4, 512], F32, name=f"ppq{half}",
                                     tag="pp")
                    for n in range(4):
                        sl = bass.ds(2048 * half + 512 * n, 512)
                        nc.tensor.matmul(ppq[:, n, :], wq_aug[:], xTr[:, sl],
                                         start=True, stop=True)
                    nc.scalar.activation(qA[:, bass.ts(half, 2048)], ppq[:],
                                         AF.Identity, scale=rstd2[:, 0:1])
                    # lo residual: (psum*rstd) - qA, fused on DVE
                    nc.vector.scalar_tensor_tensor(
                        out=qlo_t[:, bass.ts(half, 2048)],
                        in0=ppq[0:DH, :, :], scalar=rstd2[0:DH, 0:1],
                        in1=qA[0:DH, bass.ts(half, 2048)],
                        op0=ALU.mult, op1=ALU.subtract)
                    ppk = props.tile([128, 4, 512], F32, name=f"ppk{half}",
                                     tag="pp")
                    for n in range(4):
                        sl = bass.ds(2048 * half + 512 * n, 512)
                        nc.tensor.matmul(ppk[:, n, :], wk_aug[:], xTr[:, sl],
                                         start=True, stop=True)
                    nc.scalar.activation(kA[:, bass.ts(half, 2048)], ppk[:],
                                         AF.Identity, scale=rstd2[:, 1:2])
                    nc.vector.scalar_tensor_tensor(
                        out=klo_t[:, bass.ts(half, 2048)],
                        in0=ppk[0:DH, :, :], scalar=rstd2[0:DH, 1:2],
                        in1=kA[0:DH, bass.ts(half, 2048)],
                        op0=ALU.mult, op1=ALU.subtract)

                # ---- prologue-peel: pass A for chunk 0, interleaved with the
                # projection halves. 4-way row-tiled groups into one
                # [128, 2048] tile (4 banks): j-half 0 (needs only kA half 0)
                # gets an exact DVE max; j-half 1 an ACT lse bound
                # (16*ln(sum exp(s/16)) - 40 >= max-40). DVE and ACT groups
                # alternate through the PSUM ring so both engines run. ----
                l8all = prow.tile([128, 8], F32, name="l8all")
                m01all = prow.tile([128, 8], F32, name="m01all")
                bias25 = prow.tile([128, 1], F32, name="bias25")
                nc.vector.memset(bias25[:], -25.0)

                def peel_group(blk, qp, jk_pool):
                    pa = props.tile([128, 2048], F32,
                                    name=f"pa{blk}_{qp}", tag="pp")
                    for r in range(4):
                        nc.tensor.matmul(
                            pa[:, bass.ts(r, 512)],
                            qA[bass.ts(r, 32), bass.ts(blk, 128)],
                            kA[bass.ts(r, 32),
                               bass.ds(2048 * qp + 512 * r, 512)],
                            start=True, stop=True,
                            tile_position=(32 * r, 0),
                        )
                    if qp == 0:
                        nc.vector.reduce_max(m01all[:, blk:blk + 1], pa[:],
                                             axis=mybir.AxisListType.X)
                    else:
                        ju = jk_pool.tile([128, 2048], BF16,
                                          name=f"ju{blk}", tag="ju")
                        nc.scalar.activation(ju[:], pa[:], AF.Exp,
                                             scale=0.0625, bias=bias25[:],
                                             accum_out=l8all[:, blk:blk + 1])

                proj_half(0)
                if STAGE >= 2:
                    with tc.tile_pool(name="jkp", bufs=2) as jk_pool:
                        peel_group(0, 0, jk_pool)
                        peel_group(1, 0, jk_pool)
                        proj_half(1)
                        # alternate ACT-lse and DVE-max groups
                        peel_group(2, 0, jk_pool)
                        for blk in range(8):
                            peel_group(blk, 1, jk_pool)
                            if blk + 3 < 8:
                                peel_group(blk + 3, 0, jk_pool)
                    lnt = prow.tile([128, 8], F32, name="lnt")
                    nc.scalar.activation(lnt[:], l8all[:], AF.Ln)
                    mlse = prow.tile([128, 8], F32, name="mlse")
                    # m = 16*(ln l8' + 25) - 40 = 16*ln l8' + 360
                    nc.vector.tensor_scalar(out=mlse[:], in0=lnt[:],
                                            scalar1=16.0, scalar2=360.0,
                                            op0=ALU.mult, op1=ALU.add)
                    nc.vector.tensor_tensor(out=mcolT[0][:], in0=m01all[:],
                                            in1=mlse[:], op=ALU.max)
                else:
                    proj_half(1)

                # v projection -> bf16 v (rows 0:32)
                for half in range(2):
                    pv = props.tile([128, 4, 512], F32, name=f"pv{half}",
                                    tag="pp")
                    for n in range(4):
                        nc.tensor.matmul(pv[0:DH, n, :], wv_r[:],
                                         xTr[0:CIN,
                                             bass.ds(2048 * half + 512 * n,
                                                     512)],
                                         start=True, stop=True)
                    if half == 0:
                        nc.scalar.copy(vbf[:, bass.ts(half, 2048)],
                                       pv[0:DH, :, :])
                    else:
                        nc.vector.tensor_copy(vbf[:, bass.ts(half, 2048)],
                                              pv[0:DH, :, :])

                # ---- vaug: PE transposes of bf16 v -> [j, d|1] blocks ----
                nc.gpsimd.memset(vaug[:], 1.0)
                for g in range(8):
                    pt4 = props.tile([128, 4, 512], BF16, name=f"pvt{g}",
                                     tag="pp")
                    for t in range(4):
                        jb = 4 * g + t
                        nc.tensor.transpose(pt4[:, t, 0:DH],
                                            vbf[:, bass.ts(jb, 128)],
                                            identb[0:DH, 0:DH])
                    nc.vector.tensor_copy(vaug[:, 4 * g:4 * g + 4, 0:DH],
                                          pt4[:, :, 0:DH])

            # ---- hi/lo pair tiles ----
            # kP = [khi; khi; klo; -1]; qPc[c] = [qhi; qlo; qhi; mhat]
            nc.sync.dma_start(kP[0:DH, :], kA[0:DH, :])
            nc.sync.dma_start(kP[DH:2 * DH, :], kA[DH:2 * DH, :])
            nc.sync.dma_start(kP[2 * DH:3 * DH, :], klo_t[:])
            for c in range(NCH):
                cs = bass.ts(c, CHUNK)
                nc.sync.dma_start(qPc[c][0:DH, :], qA[0:DH, cs])
                nc.sync.dma_start(qPc[c][2 * DH:3 * DH, :], qA[2 * DH:3 * DH, cs])
                nc.sync.dma_start(qPc[c][DH:2 * DH, :], qlo_t[:, cs])
            # kP row 96 = -1 via tiny memset + reshape DMA
            nc.gpsimd.memset(neg1[:], -1.0)
            nc.sync.dma_start(kP[96:97, :], neg1[:])

        # ================= main loop =================
        with tc.tile_pool(name="uni", bufs=3, space="PSUM") as uni_pool, \
             tc.tile_pool(name="psAV", bufs=1, space="PSUM") as psAV_pool, \
             tc.tile_pool(name="mpp2", bufs=3) as mp_pool, \
             tc.tile_pool(name="ptp", bufs=4) as pt_pool, \
             tc.tile_pool(name="epp", bufs=2) as ep_pool:

            if STAGE <= 2:
                zout = ep_pool.tile([CIN, S], F32, name="zout", tag="zout")
                nc.vector.memset(zout[:], 0.0)
                nc.sync.dma_start(out_d[:], zout[:])

            mparts_t = {}

            def emit_passA_half(blk, qp):
                # 4-way row-tiled concurrent group covering HALF a block's
                # j range (2048 cols) via two uni slots (4 distinct banks)
                if qp == 0:
                    mparts_t[blk] = mp_pool.tile([128, 4], F32, name=f"mp{blk}",
                                                 tag="mparts")
                mp = mparts_t[blk]
                pa0 = uni_pool.tile([128, 1024], F32, name=f"pa{blk}_{qp}a",
                                    tag="uni")
                pa1 = uni_pool.tile([128, 1024], F32, name=f"pa{blk}_{qp}b",
                                    tag="uni")
                for r in range(4):
                    dst = pa0 if r < 2 else pa1
                    nc.tensor.matmul(
                        dst[:, bass.ts(r % 2, 512)],
                        qA[bass.ts(r, 32), bass.ts(blk, 128)],
                        kA[bass.ts(r, 32), bass.ds(2048 * qp + 512 * r, 512)],
                        start=True, stop=True,
                        tile_position=(32 * r, 0),
                    )
                nc.vector.reduce_max(mp[:, 2 * qp:2 * qp + 1], pa0[:],
                                     axis=mybir.AxisListType.X)
                nc.vector.reduce_max(mp[:, 2 * qp + 1:2 * qp + 2], pa1[:],
                                     axis=mybir.AxisListType.X)
                if qp == 1:
                    mparts_t.pop(blk)
                    nc.vector.reduce_max(
                        mcolT[blk // 8][:, (blk % 8):(blk % 8) + 1], mp[:],
                        axis=mybir.AxisListType.X)

            def emit_mhat(c):
                # 8 max columns -> PE transpose -> bf16 row -> reshape DMA into
                # row 96 of qPc[c]
                psm = uni_pool.tile([128, 1024], F32, name=f"psm{c}", tag="uni")
                nc.tensor.transpose(psm[0:8, 0:128], mcolT[c][:], identf[:])
                m8 = ep_pool.tile([8, 128], BF16, name=f"m8_{c}", tag="m8")
                nc.vector.tensor_copy(m8[:], psm[0:8, 0:128])
                # explicit 3D dst AP pins descriptor order (block-major)
                nc.sync.dma_start(
                    qPc[c][96:97, :].rearrange("a (b c) -> a b c", b=8), m8[:])

            avs = {}
            pts = {}

            def emit_AV_pair(c, jb0):
                # col-tiled concurrent pairs: even jb -> partitions 0:33 at
                # tile_position (0,0); odd jb -> partitions 64:97 at (0,64).
                # Adjacent matmuls must hit DIFFERENT PSUM banks to stream
                # concurrently, so interleave as (E-h0 | O-h1), (O-h0 | E-h1).
                avh = avs[c]
                pte = pts.pop(jb0)
                pto = pts.pop(jb0 + 1)
                st = (jb0 == 0)
                sp = (jb0 == NB - 2)
                nc.tensor.matmul(avh[0:DH + 1, 0:512], vaug[:, jb0, :],
                                 pte[:, 0:512], start=st, stop=sp,
                                 tile_position=(0, 0))
                nc.tensor.matmul(avh[64:64 + DH + 1, 512:1024],
                                 vaug[:, jb0 + 1, :], pto[:, 512:1024],
                                 start=st, stop=sp, tile_position=(0, 64))
                nc.tensor.matmul(avh[64:64 + DH + 1, 0:512],
                                 vaug[:, jb0 + 1, :], pto[:, 0:512],
                                 start=st, stop=sp, tile_position=(0, 64))
                nc.tensor.matmul(avh[0:DH + 1, 512:1024], vaug[:, jb0, :],
                                 pte[:, 512:1024], start=st, stop=sp,
                                 tile_position=(0, 0))

            def epilogue_steps(c):
                # chunk epilogue split into closures, one per early iteration
                # of the next chunk, to spread PSUM-ring + DVE pressure.
                # Emits the UNNORMALIZED projection wo^T @ [av; l] plus the
                # denominator row; the host divides during unshard.
                avh = avs.pop(c)
                avsb = ep_pool.tile([DH + 1, CHUNK], F32R, name=f"avsb{c}",
                                    tag="avsb", bufs=4)

                def s0():
                    # combine the two col-tiled AV halves (DVE can read only
                    # one PSUM operand per instruction)
                    nc.vector.tensor_copy(avsb[:], avh[0:DH + 1, :])
                    nc.vector.tensor_tensor(out=avsb[:], in0=avsb[:],
                                            in1=avh[64:64 + DH + 1, :],
                                            op=ALU.add)
                    nc.sync.dma_start(l_d[:, bass.ts(c, CHUNK)], avsb[32:33, :])

                def seg_step(seg):
                    def s():
                        sg = bass.ts(seg, 512)
                        psY = uni_pool.tile([128, 1024], F32, name=f"psY{c}_{seg}",
                                            tag="uni")
                        nc.tensor.matmul(psY[0:CIN, 0:512], wo_r[0:DH + 1, :],
                                         avsb[:, sg], start=True, stop=True)
                        ysb = ep_pool.tile([CIN, 512], F32, name=f"ysb{c}_{seg}",
                                           tag="ysb")
                        nc.vector.tensor_copy(ysb[:], psY[0:CIN, 0:512])
                        nc.sync.dma_start(out_d[:, bass.ds(CHUNK * c + 512 * seg,
                                                           512)], ysb[:])
                    return s

                return [s0, seg_step(0), seg_step(1)]

            if STAGE == 2:
                zout = ep_pool.tile([CIN, S], F32, name="zout", tag="zout")
                nc.vector.memset(zout[:], 0.0)
                nc.sync.dma_start(out_d[:], zout[:])
            pending = []
            for c in range(NCH if STAGE >= 3 else 0):
                if c == 0:
                    emit_mhat(0)
                if c > 0 and STAGE >= 4:
                    steps = epilogue_steps(c - 1)
                    # step 0 (avh halves -> SBUF add) must precede the
                    # reallocation of the single-buffer AV accumulator below
                    steps[0]()
                    pending = steps[1:]
                elif c > 0:
                    avs.pop(c - 1)
                avs[c] = psAV_pool.tile([128, CHUNK], F32, name=f"av{c}",
                                        tag="av")
                for jb in range(NB):
                    if c + 1 < NCH:
                        if jb % 2 == 0:
                            emit_passA_half(8 * (c + 1) + jb // 4, (jb % 4) // 2)
                        if jb == NB - 1:
                            # next chunk's mhat row, well before its pass B
                            emit_mhat(c + 1)
                    psB = uni_pool.tile([128, CHUNK], F32, name=f"psB{c}_{jb}",
                                        tag="uni")
                    for hf in range(2):
                        nc.tensor.matmul(psB[:, bass.ts(hf, 512)],
                                         kP[:, bass.ts(jb, 128)],
                                         qPc[c][:, bass.ts(hf, 512)],
                                         start=True, stop=True)
                    pt = pt_pool.tile([128, CHUNK], BF16, name=f"pt{c}_{jb}",
                                      tag="pt")
                    nc.scalar.activation(pt[:], psB[:], AF.Exp)
                    pts[jb] = pt
                    if jb >= 3 and jb % 2 == 1:
                        emit_AV_pair(c, jb - 3)
                    if pending and jb >= 2:
                        pending.pop(0)()
                emit_AV_pair(c, NB - 2)
            for step in pending:
                step()
            if STAGE >= 4:
                for step in epilogue_steps(NCH - 1):
                    step()
            elif STAGE == 3:
                avs.pop(NCH - 1)
                zout = ep_pool.tile([CIN, S], F32, name="zout", tag="zout")
                nc.vector.memset(zout[:], 0.0)
                nc.sync.dma_start(out_d[:], zout[:])

    nc.compile()
    return nc


def _get_compiled():
    global _compiled
    if _compiled is None:
        _compiled = _build()
    return _compiled


def kernel(input, w_qkv, w_out, b_out):
    import ml_dtypes
    input = np.asarray(input, dtype=np.float32)
    w_qkv = np.asarray(w_qkv, dtype=np.float32)
    w_out = np.asarray(w_out, dtype=np.float32)
    b_out = np.asarray(b_out, dtype=np.float32)
    b, x, y, z, c = input.shape
    assert (b, x, y, z, c) == (2, 16, 16, 16, 64)
    hid = HEADS * DH

    xa_by_batch = []
    for bb in range(b):
        xf = input[bb].reshape(S, CIN)
        aug = np.concatenate([xf, np.ones((S, 1), np.float32)], axis=1)
        xa = np.ascontiguousarray(
            aug.reshape(NB, 128, CIN + 1).transpose(1, 0, 2)
        ).astype(ml_dtypes.bfloat16)
        xa_by_batch.append(xa)

    in_maps = []
    xT_by_batch = []
    for bb in range(b):
        xf = input[bb].reshape(S, CIN).T
        xT_by_batch.append(np.ascontiguousarray(
            np.vstack([xf, np.ones((1, S), np.float32)])))

    for core in range(8):
        bb, h = divmod(core, HEADS)
        xT = xT_by_batch[bb]
        wq = np.tile(w_qkv[:, h * DH:(h + 1) * DH], (1, 4))
        wk = np.tile(w_qkv[:, hid + h * DH: hid + (h + 1) * DH], (1, 4))
        wv = np.ascontiguousarray(w_qkv[:, 2 * hid + h * DH: 2 * hid + (h + 1) * DH])
        wo = np.vstack([w_out[h * DH:(h + 1) * DH, :], b_out[None, :] / HEADS])
        wq = np.ascontiguousarray(wq)
        wk = np.ascontiguousarray(wk)
        in_maps.append({
            "xT": xT,
            "xa": xa_by_batch[bb],
            "wq": wq,
            "wk": wk,
            "wqf": wq.copy(),
            "wkf": wk.copy(),
            "wv": wv,
            "wo": np.ascontiguousarray(wo),
        })

    global _last_in_maps
    _last_in_maps = in_maps
    nc = _get_compiled()
    res = run_bass_kernel_spmd(nc, in_maps, core_ids=list(range(8)))
    out = np.zeros((b, S, CIN), dtype=np.float32)
    for core in range(8):
        bb = core // HEADS
        num = res.results[core]["out"]          # [64, S], unnormalized
        l = res.results[core]["ldenom"][0]      # [S]
        out[bb] += (num / l[None, :]).T
    return out.reshape(b, x, y, z, CIN)


if __name__ == "__main__":
    rng = np.random.default_rng(0)
    inp = rng.standard_normal((2, 16, 16, 16, 64), dtype=np.float32)
    wqkv = rng.standard_normal((64, 384), dtype=np.float32) / 8.0
    wout = rng.standard_normal((128, 64), dtype=np.float32) / np.sqrt(128)
    bout = np.zeros(64, dtype=np.float32)
    o = kernel(inp, wqkv, wout, bout)
    print("kernel output shape:", o.shape)
